# revision 19
# baseline (speedup 1.0000x reference)
"""Trainium2 Bass kernel for BinSpecCNN: 7x (BN -> sign) + 6 binary convs
with maxpool + final FC, data-parallel over 8 NeuronCores (8 samples each).

Strategy per core:
  - bn_i + sign fused as ACT Sign(scale*x+bias) passes; batch statistics
    (per-channel sum/sumsq) all-reduced across cores per layer.
  - convs: bf16 matmuls (inputs/weights are exactly +-1 in bf16), contraction
    dim = (sample-blockdiag x Cin x KH) on partitions, KW looped via
    free-dim-shifted rhs with PSUM accumulation. Input rows are KH-replicated
    into SBUF by DMA.
  - maxpool: two InstPool stages (horizontal window then vertical) on f16.
  - conv outputs (integer-valued, |v| <= 800) stored in f16 (exact).
"""

import sys

sys.path.insert(0, "/opt/trn_rl_repo")

import numpy as np

import concourse.bass as bass
import concourse.mybir as mybir
import concourse.tile as tile
from concourse import bacc
from concourse.bass_utils import run_bass_kernel_spmd

F32 = mybir.dt.float32
F16 = mybir.dt.float16
BF16 = mybir.dt.bfloat16

N_CORES = 8
NS = 8  # samples per core
EPS = 1e-5

# (Cin, Cout, KH, KW, Hin, pool_stride, sample_group)
CONVS = [
    (3, 8, 11, 11, 224, 1, 2),
    (8, 16, 7, 7, 212, 2, 2),
    (16, 32, 5, 5, 102, 2, 1),
    (32, 32, 5, 5, 48, 2, 1),
    (32, 64, 5, 5, 21, 2, 1),
    (64, 64, 3, 3, 8, 2, 1),
]


def conv_dims(li):
    cin, cout, kh, kw, hin, ps, sg = CONVS[li]
    ch = hin - kh + 1            # conv output height/width
    oh = (ch - 3) // ps + 1      # pooled output height/width
    return cin, cout, kh, kw, hin, ps, sg, ch, oh


def kh_splits(cin, kh):
    """Split KH taps so each split's (cin*nkh) <= 128 partitions."""
    max_kh = 128 // cin
    splits = []
    k0 = 0
    while k0 < kh:
        n = min(max_kh, kh - k0)
        splits.append((k0, n))
        k0 += n
    return splits


# rows-of-conv-output per psum tile (free dim = ry*cw <= 512)
def psum_rows(cw):
    return max(1, min(512 // cw, 8))


# ---------------------------------------------------------------------------
# host-side weight packing
# ---------------------------------------------------------------------------

def pack_conv_weights(w, sg):
    """w: [Cout, Cin, KH, KW] float32 -> per-khsplit lhsT arrays
    [K, KW, M] bf16 with K=(s, c, kh), M=(s, co) block-diagonal over s."""
    wb = np.sign(w).astype(np.float32)
    cout, cin, kh, kw = w.shape
    out = []
    for (k0, nkh) in kh_splits(cin, kh):
        m = sg * cout
        k = sg * cin * nkh
        arr = np.zeros((k, kw, m), np.float32)
        for s in range(sg):
            for c in range(cin):
                for kk in range(nkh):
                    p = (s * cin + c) * nkh + kk
                    # lhsT[p, kw, s*cout + co] = wb[co, c, k0+kk, kw]
                    arr[p, :, s * cout:(s + 1) * cout] = wb[:, c, k0 + kk, :].T
        out.append(ml_bf16(arr))
    return out


def ml_bf16(a):
    import ml_dtypes
    return np.asarray(a, dtype=np.float32).astype(ml_dtypes.bfloat16)


def ml_f16(a):
    return np.asarray(a, dtype=np.float16)


def make_selector(c, nb):
    """[128, c] selector: partition p=(g*c+ch) -> column ch."""
    sel = np.zeros((128, c), np.float32)
    for ch in range(c):
        for g in range(nb):
            sel[g * c + ch, ch] = 1.0
    return sel


# ---------------------------------------------------------------------------
# device kernel builder
# ---------------------------------------------------------------------------

def build(nc):
    x_in = nc.declare_dram_parameter("x", [NS, 3, 224, 224], F32, isOutput=False)
    out_fc = nc.declare_dram_parameter("out", [NS, 50], F32, isOutput=True)

    # weights / constants (identical on all cores)
    w_params = []   # per layer: list of lhsT dram handles
    for li in range(6):
        cin, cout, kh, kw, hin, ps, sg, ch, oh = conv_dims(li)
        splits = kh_splits(cin, kh)
        handles = []
        for si, (k0, nkh) in enumerate(splits):
            k = sg * cin * nkh
            handles.append(nc.declare_dram_parameter(
                f"w{li + 1}_{si}", [k, kw, sg * cout], BF16, isOutput=False))
        w_params.append(handles)
    wfcT = nc.declare_dram_parameter("wfcT", [256, 50], BF16, isOutput=False)

    gb = [nc.declare_dram_parameter(f"gb{i}", [c, 2], F32, isOutput=False)
          for i, c in zip(range(1, 8), [3, 8, 16, 32, 32, 64, 64])]
    sel_in = nc.declare_dram_parameter("sels", [128, 3 + 8 + 16 + 32 + 32 + 64 + 64],
                                       F32, isOutput=False)

    # internal DRAM
    a_act = []   # binarized activations bf16, input of conv li
    for li in range(6):
        cin, cout, kh, kw, hin, ps, sg, ch, oh = conv_dims(li)
        a_act.append(nc.dram_tensor(f"act{li}", [NS, cin, hin, hin], BF16))
    convout = []
    pooled = []
    for li in range(6):
        cin, cout, kh, kw, hin, ps, sg, ch, oh = conv_dims(li)
        convout.append(nc.dram_tensor(f"conv{li}", [NS, cout, ch, ch], F16))
        pooled.append(nc.dram_tensor(f"pool{li}", [NS, cout, oh, oh], F16))
    h_bin = nc.dram_tensor("h_bin", [NS, 64, 2, 2], BF16)  # final binarized

    cc_in = [nc.dram_tensor(f"cc_in{i}", [c, 2], F32)
             for i, c in zip(range(7), [3, 8, 16, 32, 32, 64, 64])]
    cc_out = [nc.dram_tensor(f"cc_out{i}", [c, 2], F32, addr_space="Shared")
              for i, c in zip(range(7), [3, 8, 16, 32, 32, 64, 64])]
    st_dram = [nc.dram_tensor(f"st{i}", [c, 2], F32)
               for i, c in zip(range(7), [3, 8, 16, 32, 32, 64, 64])]

    with tile.TileContext(nc) as tc:
        build_body(nc, tc, x_in, out_fc, w_params, wfcT, gb, sel_in,
                   a_act, convout, pooled, h_bin, cc_in, cc_out, st_dram)
    nc.finalize()
    return nc


def dmaap(t, offset, dims):
    """AP over dram tensor handle t with element offset and [step,count] dims."""
    base = t[tuple(slice(None) for _ in t.shape)]
    return bass.AP(tensor=base.tensor, offset=offset, ap=[list(d) for d in dims])


def build_body(nc, tc, x_in, out_fc, w_params, wfcT, gb, sel_in,
               a_act, convout, pooled, h_bin, cc_in, cc_out, st_dram):
    import contextlib
    ctx = contextlib.ExitStack()
    singles = ctx.enter_context(tc.tile_pool(name="singles", bufs=1))
    stats_pool = ctx.enter_context(tc.tile_pool(name="stats", bufs=1))
    small = ctx.enter_context(tc.tile_pool(name="small", bufs=4))

    # ---- load all weights once ----
    w_sb = []
    for li in range(6):
        cin, cout, kh, kw, hin, ps, sg, ch, oh = conv_dims(li)
        tiles = []
        for si, (k0, nkh) in enumerate(kh_splits(cin, kh)):
            k = sg * cin * nkh
            wt = singles.tile([k, kw, sg * cout], BF16, tag=f"w{li}_{si}")
            nc.sync.dma_start(out=wt, in_=w_params[li][si][:, :, :])
            tiles.append(wt)
        w_sb.append(tiles)
    wfc_sb = singles.tile([128, 2, 50], BF16, tag="wfc")
    nc.sync.dma_start(out=wfc_sb, in_=dmaap(wfcT, 0, [[50, 128], [6400, 2], [1, 50]]))
    sel_sb = singles.tile([128, 219], F32, tag="sel")
    nc.sync.dma_start(out=sel_sb, in_=sel_in[:, :])
    gb_sb = [singles.tile([c, 2], F32, tag=f"gb{i}", name=f"gb_sb{i}")
             for i, c in zip(range(7), [3, 8, 16, 32, 32, 64, 64])]
    for i in range(7):
        nc.sync.dma_start(out=gb_sb[i], in_=gb[i][:, :])
    eps_sb = singles.tile([64, 1], F32, tag="eps")
    nc.vector.memset(eps_sb, EPS)

    # HAM "heater": K=128 matmuls are the only shape that un-throttles the
    # PE clock (1.2 -> 2.4 GHz); small-K conv matmuls never trigger it.
    # Issue one every ~2us so the whole kernel runs warm.
    w_warm = singles.tile([128, 64], BF16, tag="w_warm")
    nc.vector.memset(w_warm, 1.0)
    heat_pool = ctx.enter_context(tc.tile_pool(name="heat", bufs=1, space="PSUM"))
    heat_ps = heat_pool.tile([64, 64], F32, tag="heat")

    def heater():
        nc.tensor.matmul(heat_ps, w_warm[:, 0:64], w_warm[:, 0:64],
                         start=True, stop=True)

    sel_off = [0, 3, 11, 27, 59, 91, 155]  # column offsets per layer in sel_sb
    sel_w = [3, 8, 16, 32, 32, 64, 64]

    def stats_finish(li, acc, n_tot):
        """acc: [128, 2] sbuf partials -> allreduce -> s,t in st_dram[li].
        li: 0..6 (bn index). Returns nothing; st_dram[li] written."""
        c = sel_w[li]
        with tc.tile_pool(name=f"stp{li}", bufs=1, space="PSUM") as psp:
            ps = psp.tile([c, 2], F32)
            nc.tensor.matmul(ps, sel_sb[:, sel_off[li]:sel_off[li] + c], acc,
                             start=True, stop=True)
            red = small.tile([c, 2], F32, tag="red")
            nc.vector.tensor_copy(red, ps)
        nc.sync.dma_start(out=cc_in[li][:, :], in_=red)
        nc.gpsimd.collective_compute(
            "AllReduce", mybir.AluOpType.add,
            replica_groups=[list(range(N_CORES))],
            ins=[cc_in[li][:, :]], outs=[cc_out[li][:, :]])
        tot = small.tile([c, 2], F32, tag="tot")
        nc.sync.dma_start(out=tot, in_=cc_out[li][:, :])
        # mean/ex2
        mv = small.tile([c, 2], F32, tag="mv")
        nc.scalar.activation(mv, tot, mybir.ActivationFunctionType.Copy,
                             bias=0.0, scale=1.0 / n_tot)
        m2 = small.tile([c, 1], F32, tag="m2")
        nc.scalar.square(m2, mv[:, 0:1])
        var = small.tile([c, 1], F32, tag="var")
        nc.vector.tensor_sub(var, mv[:, 1:2], m2)
        sd = small.tile([c, 1], F32, tag="sd")
        nc.scalar.activation(sd, var, mybir.ActivationFunctionType.Sqrt,
                             bias=eps_sb[0:c, :], scale=1.0)
        rstd = small.tile([c, 1], F32, tag="rstd")
        nc.vector.reciprocal(rstd, sd)
        st = small.tile([c, 2], F32, tag="st")
        nc.vector.tensor_mul(st[:, 0:1], gb_sb[li][:, 0:1], rstd)       # s
        ms = small.tile([c, 1], F32, tag="ms")
        nc.vector.tensor_mul(ms, mv[:, 0:1], st[:, 0:1])
        nc.vector.tensor_sub(st[:, 1:2], gb_sb[li][:, 1:2], ms)         # t
        nc.sync.dma_start(out=st_dram[li][:, :], in_=st)

    # ================= bn1 over x, binarize -> act0 =================
    # partition p = c*32 + n*4 + yc  (c:3, n:8, yc:4 chunks of 56 rows),
    # free chunked in 4 pieces of 14 rows to bound SBUF usage.
    XF = 14 * 224
    acc0 = stats_pool.tile([128, 2], F32, tag="acc0")
    nc.vector.memset(acc0, 0.0)
    with tc.tile_pool(name="bn1", bufs=2) as bn1p:
        for ci in range(4):
            heater()
            xt = bn1p.tile([96, XF], F32, tag="xt")
            off = ci * XF
            nc.sync.dma_start(out=xt, in_=dmaap(
                x_in, off, [[4 * XF, 96], [1, XF]]))
            r0 = bn1p.tile([96, 1], F32, tag="r0")
            nc.vector.tensor_reduce(out=r0, in_=xt,
                                    axis=mybir.AxisListType.X,
                                    op=mybir.AluOpType.add)
            nc.vector.tensor_add(acc0[0:96, 0:1], acc0[0:96, 0:1], r0)
            sq0 = bn1p.tile([96, XF], BF16, tag="sq0")
            r1 = bn1p.tile([96, 1], F32, tag="r1")
            nc.scalar.activation(sq0, xt, mybir.ActivationFunctionType.Square,
                                 accum_out=r1)
            nc.vector.tensor_add(acc0[0:96, 1:2], acc0[0:96, 1:2], r1)
        stats_finish(0, acc0, 64 * 224 * 224)
        # broadcast s,t -> [96, 2] and sign
        st0 = bn1p.tile([96, 2], F32, tag="st0")
        st0b = st0[:, :]
        for yc in range(4):
            # partitions p = n*12 + c*4 + yc for (n, c); partition pitch = 2
            dst = bass.AP(tensor=st0b.tensor, offset=st0b.offset + yc * st0b.ap[0][0],
                          ap=[[4 * st0b.ap[0][0], 24], [1, 2]])
            nc.sync.dma_start(out=dst,
                              in_=dmaap(st_dram[0], 0, [[0, 8], [2, 3], [1, 2]]))
        for ci in range(4):
            xt = bn1p.tile([96, XF], F32, tag="xt")
            off = ci * XF
            nc.sync.dma_start(out=xt, in_=dmaap(
                x_in, off, [[4 * XF, 96], [1, XF]]))
            bin0 = bn1p.tile([96, XF], BF16, tag="bin0")
            nc.scalar.activation(bin0, xt, mybir.ActivationFunctionType.Sign,
                                 bias=st0[:, 1:2], scale=st0[:, 0:1])
            nc.sync.dma_start(
                out=dmaap(a_act[0], off, [[4 * XF, 96], [1, XF]]),
                in_=bin0)

    # ================= conv layers =================
    for li in range(6):
        cin, cout, kh, kw, hin, ps, sg, ch, oh = conv_dims(li)
        cw = ch
        win = hin
        splits = kh_splits(cin, kh)
        m = sg * cout
        ry = psum_rows(cw)
        yb = 16 if ch > 16 else ch
        yb = (yb // ry) * ry if ch > 16 else ch

        with tc.tile_pool(name=f"c{li}R", bufs=3) as rpool, \
             tc.tile_pool(name=f"c{li}P", bufs=6, space="PSUM") as pspool, \
             tc.tile_pool(name=f"c{li}S", bufs=3) as stpool:
            for g in range(NS // sg):
                n0 = g * sg
                y0 = 0
                while y0 < ch:
                    nb_rows = min(yb, ch - y0)
                    # load R tiles (kh-replicated input rows)
                    rts = []
                    for si, (k0, nkh) in enumerate(splits):
                        kdim = sg * cin * nkh
                        rt = rpool.tile([kdim, nb_rows, win], BF16,
                                        tag=f"R{si}")
                        src = dmaap(
                            a_act[li],
                            n0 * cin * win * win + (y0 + k0) * win,
                            [[win * win, sg * cin],
                             [win, nkh], [1, nb_rows * win]])
                        nc.sync.dma_start(out=rt, in_=src)
                        rts.append(rt)
                    ry0 = 0
                    while ry0 < nb_rows:
                        heater()
                        nr = min(ry, nb_rows - ry0)
                        pst = pspool.tile([m, ry, cw], F32, tag="ps")
                        first = True
                        for si, (k0, nkh) in enumerate(splits):
                            for kx in range(kw):
                                nc.tensor.matmul(
                                    pst[:, 0:nr, :],
                                    w_sb[li][si][:, kx, :],
                                    rts[si][:, ry0:ry0 + nr, kx:kx + cw],
                                    start=first,
                                    stop=(si == len(splits) - 1 and kx == kw - 1))
                                first = False
                        stg = stpool.tile([m, ry, cw], F16, tag="stg")
                        if (ry0 // ry) % 2 == 0:
                            nc.scalar.copy(stg[:, 0:nr, :], pst[:, 0:nr, :])
                        else:
                            nc.vector.tensor_copy(stg[:, 0:nr, :], pst[:, 0:nr, :])
                        # store to convout dram
                        dst = dmaap(
                            convout[li],
                            n0 * cout * ch * cw + (y0 + ry0) * cw,
                            [[ch * cw, sg * cout], [1, nr * cw]])
                        nc.sync.dma_start(out=dst, in_=stg[:, 0:nr, :])
                        ry0 += nr
                    y0 += nb_rows

        # ---- pool + stats pass ----
        nb = min(NS, 128 // cout)      # samples per pass
        p_used = cout * nb
        ox = oh
        acc = stats_pool.tile([128, 2], F32, tag=f"acc{li + 1}")
        nc.vector.memset(acc, 0.0)
        # chunk pooled rows
        oyc = oh
        while (ps * (oyc - 1) + 3) * cw * 2 > 12000:
            oyc = (oyc + 1) // 2
        max_rows = ps * (oyc - 1) + 3
        with tc.tile_pool(name=f"p{li}", bufs=2) as ppool:
            for pa in range(NS // nb):
                npa = pa * nb
                oy0 = 0
                while oy0 < oh:
                    heater()
                    noy = min(oyc, oh - oy0)
                    rows = ps * (noy - 1) + 3
                    ld = ppool.tile([p_used, max_rows, cw], F16, tag="ld")
                    src = dmaap(
                        convout[li],
                        npa * cout * ch * cw + (oy0 * ps) * cw,
                        [[ch * cw, nb * cout], [1, rows * cw]])
                    nc.sync.dma_start(out=ld[:, 0:rows, :], in_=src)
                    ldb = ld[:, :, :]

                    def ldv(dx):
                        return bass.AP(
                            tensor=ldb.tensor, offset=ldb.offset + dx,
                            ap=[[ldb.ap[0][0], p_used], [cw, rows], [ps, ox]])

                    ph = ppool.tile([p_used, max_rows, ox], F16, tag="ph")
                    nc.vector.tensor_max(ph[:, 0:rows, :], ldv(0), ldv(1))
                    nc.vector.tensor_max(ph[:, 0:rows, :], ph[:, 0:rows, :], ldv(2))
                    phb = ph[:, :, :]

                    def phv(dy):
                        return bass.AP(
                            tensor=phb.tensor, offset=phb.offset + dy * ox,
                            ap=[[phb.ap[0][0], p_used], [ps * ox, noy], [1, ox]])

                    pv = ppool.tile([p_used, oyc, ox], F16, tag="pv")
                    nc.vector.tensor_max(pv[:, 0:noy, :], phv(0), phv(1))
                    nc.vector.tensor_max(pv[:, 0:noy, :], pv[:, 0:noy, :], phv(2))
                    # stats
                    red = ppool.tile([p_used, 1], F32, tag="redp")
                    nc.vector.tensor_reduce(out=red, in_=pv[:, 0:noy, :],
                                            axis=mybir.AxisListType.XY,
                                            op=mybir.AluOpType.add)
                    nc.vector.tensor_add(acc[0:p_used, 0:1], acc[0:p_used, 0:1], red)
                    sqs = ppool.tile([p_used, oyc, ox], BF16, tag="sqs")
                    red2 = ppool.tile([p_used, 1], F32, tag="redq")
                    nc.scalar.activation(sqs[:, 0:noy, :], pv[:, 0:noy, :],
                                         mybir.ActivationFunctionType.Square,
                                         accum_out=red2)
                    nc.vector.tensor_add(acc[0:p_used, 1:2], acc[0:p_used, 1:2], red2)
                    # store pooled
                    dst = dmaap(
                        pooled[li],
                        npa * cout * oh * ox + oy0 * ox,
                        [[oh * ox, nb * cout], [1, noy * ox]])
                    nc.sync.dma_start(out=dst, in_=pv[:, 0:noy, :])
                    oy0 += noy

        stats_finish(li + 1, acc, 64 * oh * oh)

        # ---- binarize pass: pooled -> a_act[li+1] (or h_bin for last) ----
        dst_t = a_act[li + 1] if li < 5 else h_bin
        stb = stats_pool.tile([128, 2], F32, tag=f"stb{li + 1}")
        nc.sync.dma_start(out=stb[0:p_used, :],
                          in_=dmaap(st_dram[li + 1], 0, [[0, nb], [2, cout], [1, 2]]))
        f_all = oh * oh
        with tc.tile_pool(name=f"b{li}", bufs=3) as bpool:
            for pa in range(NS // nb):
                npa = pa * nb
                f0 = 0
                fchunk = min(f_all, 4096)
                while f0 < f_all:
                    heater()
                    nf = min(fchunk, f_all - f0)
                    ld = bpool.tile([p_used, fchunk], F16, tag="bl")
                    nc.sync.dma_start(
                        out=ld[:, 0:nf],
                        in_=dmaap(pooled[li], npa * cout * f_all + f0,
                                  [[f_all, nb * cout], [1, nf]]))
                    bo = bpool.tile([p_used, fchunk], BF16, tag="bo")
                    nc.scalar.activation(bo[:, 0:nf], ld[:, 0:nf],
                                         mybir.ActivationFunctionType.Sign,
                                         bias=stb[0:p_used, 1:2],
                                         scale=stb[0:p_used, 0:1])
                    nc.sync.dma_start(
                        out=dmaap(dst_t, npa * cout * f_all + f0,
                                  [[f_all, nb * cout], [1, nf]]),
                        in_=bo[:, 0:nf])
                    f0 += nf

    # ================= FC =================
    with tc.tile_pool(name="fc", bufs=1) as fcp, \
         tc.tile_pool(name="fcps", bufs=1, space="PSUM") as fcps:
        ht = [fcp.tile([128, NS], BF16, tag=f"ht{i}", name=f"ht{i}") for i in range(2)]
        for i in range(2):
            nc.sync.dma_start(out=ht[i], in_=dmaap(
                h_bin, i * 128, [[1, 128], [256, NS]]))
        ps = fcps.tile([NS, 50], F32)
        nc.tensor.matmul(ps, ht[0], wfc_sb[:, 0, :], start=True, stop=False)
        nc.tensor.matmul(ps, ht[1], wfc_sb[:, 1, :], start=False, stop=True)
        fo = fcp.tile([NS, 50], F32, tag="fo")
        nc.vector.tensor_copy(fo, ps)
        nc.sync.dma_start(out=out_fc[:, :], in_=fo)

    ctx.close()


# ---------------------------------------------------------------------------
# entry point
# ---------------------------------------------------------------------------

_cache = {}


def _get_nc():
    if "nc" not in _cache:
        nc = bacc.Bacc("TRN2", target_bir_lowering=False, num_devices=N_CORES)
        _cache["nc"] = build(nc)
    return _cache["nc"]


def _shared_maps(inputs):
    shared = {}
    for li in range(6):
        w = np.asarray(inputs[f"w{li + 1}"], np.float32)
        cin, cout, kh, kw, hin, ps, sg, ch, oh = conv_dims(li)
        for si, arr in enumerate(pack_conv_weights(w, sg)):
            shared[f"w{li + 1}_{si}"] = arr
    shared["wfcT"] = ml_bf16(np.sign(np.asarray(inputs["wfc"], np.float32)).T.copy())
    for i, c in zip(range(1, 8), [3, 8, 16, 32, 32, 64, 64]):
        shared[f"gb{i}"] = np.stack([np.asarray(inputs[f"g{i}"], np.float32),
                                     np.asarray(inputs[f"b{i}"], np.float32)],
                                    axis=1).copy()
    sels = []
    for c in [3, 8, 16, 32, 32, 64, 64]:
        if c == 3:
            sel = np.zeros((128, 3), np.float32)
            for ch_ in range(3):
                for n in range(NS):
                    for yc in range(4):
                        sel[n * 12 + ch_ * 4 + yc, ch_] = 1.0
        else:
            sel = make_selector(c, min(NS, 128 // c))
        sels.append(sel)
    shared["sels"] = np.concatenate(sels, axis=1).copy()
    return shared


def _in_maps(inputs):
    x = np.asarray(inputs["x"], np.float32)
    if "shared" not in _cache:
        _cache["shared"] = _shared_maps(inputs)
    shared = _cache["shared"]
    in_maps = []
    for c in range(N_CORES):
        m = dict(shared)
        m["x"] = np.ascontiguousarray(x[c * NS:(c + 1) * NS])
        in_maps.append(m)
    return in_maps


def _make_runner(nc):
    """Cached jitted SPMD executor (run_bass_via_pjrt retraces per call)."""
    import jax
    from jax.experimental.shard_map import shard_map
    from jax.sharding import Mesh, PartitionSpec
    from concourse import bass2jax, mybir as _mb

    bass2jax.install_neuronx_cc_hook()
    partition_name = (nc.partition_id_tensor.name
                      if nc.partition_id_tensor else None)
    in_names, out_names, out_avals, zero_outs = [], [], [], []
    for alloc in nc.m.functions[0].allocations:
        if not isinstance(alloc, _mb.MemoryLocationSet):
            continue
        name = alloc.memorylocations[0].name
        if alloc.kind == "ExternalInput":
            if name != partition_name:
                in_names.append(name)
        elif alloc.kind == "ExternalOutput":
            out_names.append(name)
            shape = tuple(alloc.tensor_shape)
            dtype = _mb.dt.np(alloc.dtype)
            out_avals.append(jax.core.ShapedArray(shape, dtype))
            zero_outs.append(np.zeros((N_CORES * shape[0],) + shape[1:], dtype))
    n_params = len(in_names)
    all_names = in_names + out_names
    if partition_name is not None:
        all_names = all_names + [partition_name]
    donate = tuple(range(n_params, n_params + len(out_names)))

    def _body(*args):
        operands = list(args)
        if partition_name is not None:
            operands.append(bass2jax.partition_id_tensor())
        outs = bass2jax._bass_exec_p.bind(
            *operands, out_avals=tuple(out_avals), in_names=tuple(all_names),
            out_names=tuple(out_names), lowering_input_output_aliases=(),
            sim_require_finite=True, sim_require_nnan=True, nc=nc)
        return tuple(outs)

    devices = jax.devices()[:N_CORES]
    mesh = Mesh(np.asarray(devices), ("core",))
    nin = n_params + len(out_names)
    sharded = jax.jit(
        shard_map(_body, mesh=mesh, in_specs=(PartitionSpec("core"),) * nin,
                  out_specs=(PartitionSpec("core"),) * len(out_names),
                  check_rep=False),
        donate_argnums=donate, keep_unused=True)

    def run(in_maps):
        concat_in = [np.concatenate([np.asarray(in_maps[c][n])
                                     for c in range(N_CORES)], axis=0)
                     for n in in_names]
        out_arrs = sharded(*concat_in, *zero_outs)
        return np.asarray(out_arrs[out_names.index("out")])

    return run


def kernel(**inputs):
    nc = _get_nc()
    if "runner" not in _cache:
        _cache["runner"] = _make_runner(nc)
    return _cache["runner"](_in_maps(inputs))


def profile(**inputs):
    """Run with NTFF tracing; returns (exec_time_ns, results) or None."""
    import types
    try:
        from antenv import axon_hooks
    except ImportError:
        import antenv
        axon_hooks = types.ModuleType("antenv.axon_hooks")
        axon_hooks._hook = None
        def _set(h):
            axon_hooks._hook = h
        def _get():
            return axon_hooks._hook
        axon_hooks.set_axon_ntff_profile_hook = _set
        axon_hooks.get_axon_ntff_profile_hook = _get
        sys.modules["antenv.axon_hooks"] = axon_hooks
        antenv.axon_hooks = axon_hooks
    if axon_hooks.get_axon_ntff_profile_hook() is None:
        sys.path.insert(0, "/root/.axon_site")
        from trn_agent_boot.trn_boot import _ntff_profile_via_ctypes
        hook = _ntff_profile_via_ctypes("/opt/axon/libaxon_pjrt.so")
        if hook is not None:
            axon_hooks.set_axon_ntff_profile_hook(hook)
    import concourse.bass_utils as bu
    bu.upload_artifacts = lambda tmpdir: "(skipped)"
    nc = _get_nc()
    res = run_bass_kernel_spmd(nc, _in_maps(inputs), list(range(N_CORES)),
                               trace=True)
    return res.exec_time_ns


if __name__ == "__main__":
    rng = np.random.default_rng(0)
    fake = {"x": rng.standard_normal((64, 3, 224, 224), dtype=np.float32)}
    for i, (ci, co, kh, kw) in zip(range(1, 7),
                                   [(3, 8, 11, 11), (8, 16, 7, 7), (16, 32, 5, 5),
                                    (32, 32, 5, 5), (32, 64, 5, 5), (64, 64, 3, 3)]):
        fake[f"w{i}"] = rng.standard_normal((co, ci, kh, kw), dtype=np.float32) * 0.1
    for i, c in zip(range(1, 8), [3, 8, 16, 32, 32, 64, 64]):
        fake[f"g{i}"] = np.ones(c, np.float32)
        fake[f"b{i}"] = np.zeros(c, np.float32)
    fake["wfc"] = rng.standard_normal((50, 256), dtype=np.float32) * 0.1
    out = kernel(**fake)
    print("out", out.shape, out[0, :5])


# revision 20
# speedup vs baseline: 1.0031x; 1.0031x over previous
"""Trainium2 Bass kernel for BinSpecCNN: 7x (BN -> sign) + 6 binary convs
with maxpool + final FC, data-parallel over 8 NeuronCores (8 samples each).

Strategy per core:
  - bn_i + sign fused as ACT Sign(scale*x+bias) passes; batch statistics
    (per-channel sum/sumsq) all-reduced across cores per layer.
  - convs: bf16 matmuls (inputs/weights are exactly +-1 in bf16), contraction
    dim = (sample-blockdiag x Cin x KH) on partitions, KW looped via
    free-dim-shifted rhs with PSUM accumulation. Input rows are KH-replicated
    into SBUF by DMA.
  - maxpool: two InstPool stages (horizontal window then vertical) on f16.
  - conv outputs (integer-valued, |v| <= 800) stored in f16 (exact).
"""

import sys

sys.path.insert(0, "/opt/trn_rl_repo")

import numpy as np

import concourse.bass as bass
import concourse.mybir as mybir
import concourse.tile as tile
from concourse import bacc
from concourse.bass_utils import run_bass_kernel_spmd

F32 = mybir.dt.float32
F16 = mybir.dt.float16
BF16 = mybir.dt.bfloat16

N_CORES = 8
NS = 8  # samples per core
EPS = 1e-5

# (Cin, Cout, KH, KW, Hin, pool_stride, sample_group)
CONVS = [
    (3, 8, 11, 11, 224, 1, 2),
    (8, 16, 7, 7, 212, 2, 2),
    (16, 32, 5, 5, 102, 2, 1),
    (32, 32, 5, 5, 48, 2, 1),
    (32, 64, 5, 5, 21, 2, 1),
    (64, 64, 3, 3, 8, 2, 1),
]


def conv_dims(li):
    cin, cout, kh, kw, hin, ps, sg = CONVS[li]
    ch = hin - kh + 1            # conv output height/width
    oh = (ch - 3) // ps + 1      # pooled output height/width
    return cin, cout, kh, kw, hin, ps, sg, ch, oh


def kh_splits(cin, kh):
    """Split KH taps so each split's (cin*nkh) <= 128 partitions."""
    max_kh = 128 // cin
    splits = []
    k0 = 0
    while k0 < kh:
        n = min(max_kh, kh - k0)
        splits.append((k0, n))
        k0 += n
    return splits


# rows-of-conv-output per psum tile (free dim = ry*cw <= 512)
def psum_rows(cw):
    return max(1, min(512 // cw, 8))


# ---------------------------------------------------------------------------
# host-side weight packing
# ---------------------------------------------------------------------------

def pack_conv_weights(w, sg):
    """w: [Cout, Cin, KH, KW] float32 -> per-khsplit lhsT arrays
    [K, KW, M] bf16 with K=(s, c, kh), M=(s, co) block-diagonal over s."""
    wb = np.sign(w).astype(np.float32)
    cout, cin, kh, kw = w.shape
    out = []
    for (k0, nkh) in kh_splits(cin, kh):
        m = sg * cout
        k = sg * cin * nkh
        arr = np.zeros((k, kw, m), np.float32)
        for s in range(sg):
            for c in range(cin):
                for kk in range(nkh):
                    p = (s * cin + c) * nkh + kk
                    # lhsT[p, kw, s*cout + co] = wb[co, c, k0+kk, kw]
                    arr[p, :, s * cout:(s + 1) * cout] = wb[:, c, k0 + kk, :].T
        out.append(ml_bf16(arr))
    return out


def ml_bf16(a):
    import ml_dtypes
    return np.asarray(a, dtype=np.float32).astype(ml_dtypes.bfloat16)


def ml_f16(a):
    return np.asarray(a, dtype=np.float16)


def make_selector(c, nb):
    """[128, c] selector: partition p=(g*c+ch) -> column ch."""
    sel = np.zeros((128, c), np.float32)
    for ch in range(c):
        for g in range(nb):
            sel[g * c + ch, ch] = 1.0
    return sel


# ---------------------------------------------------------------------------
# device kernel builder
# ---------------------------------------------------------------------------

def build(nc):
    x_in = nc.declare_dram_parameter("x", [NS, 3, 224, 224], F32, isOutput=False)
    out_fc = nc.declare_dram_parameter("out", [NS, 50], F32, isOutput=True)

    # weights / constants (identical on all cores)
    w_params = []   # per layer: list of lhsT dram handles
    for li in range(6):
        cin, cout, kh, kw, hin, ps, sg, ch, oh = conv_dims(li)
        splits = kh_splits(cin, kh)
        handles = []
        for si, (k0, nkh) in enumerate(splits):
            k = sg * cin * nkh
            handles.append(nc.declare_dram_parameter(
                f"w{li + 1}_{si}", [k, kw, sg * cout], BF16, isOutput=False))
        w_params.append(handles)
    wfcT = nc.declare_dram_parameter("wfcT", [256, 50], BF16, isOutput=False)

    gb = [nc.declare_dram_parameter(f"gb{i}", [c, 2], F32, isOutput=False)
          for i, c in zip(range(1, 8), [3, 8, 16, 32, 32, 64, 64])]
    sel_in = nc.declare_dram_parameter("sels", [128, 3 + 8 + 16 + 32 + 32 + 64 + 64],
                                       F32, isOutput=False)

    # internal DRAM
    a_act = []   # binarized activations bf16, input of conv li
    for li in range(6):
        cin, cout, kh, kw, hin, ps, sg, ch, oh = conv_dims(li)
        a_act.append(nc.dram_tensor(f"act{li}", [NS, cin, hin, hin], BF16))
    convout = []
    pooled = []
    for li in range(6):
        cin, cout, kh, kw, hin, ps, sg, ch, oh = conv_dims(li)
        convout.append(nc.dram_tensor(f"conv{li}", [NS, cout, ch, ch], F16))
        pooled.append(nc.dram_tensor(f"pool{li}", [NS, cout, oh, oh], F16))
    h_bin = nc.dram_tensor("h_bin", [NS, 64, 2, 2], BF16)  # final binarized

    cc_in = [nc.dram_tensor(f"cc_in{i}", [c, 2], F32)
             for i, c in zip(range(7), [3, 8, 16, 32, 32, 64, 64])]
    cc_out = [nc.dram_tensor(f"cc_out{i}", [c, 2], F32, addr_space="Shared")
              for i, c in zip(range(7), [3, 8, 16, 32, 32, 64, 64])]
    st_dram = [nc.dram_tensor(f"st{i}", [c, 2], F32)
               for i, c in zip(range(7), [3, 8, 16, 32, 32, 64, 64])]

    with tile.TileContext(nc) as tc:
        build_body(nc, tc, x_in, out_fc, w_params, wfcT, gb, sel_in,
                   a_act, convout, pooled, h_bin, cc_in, cc_out, st_dram)
    nc.finalize()
    return nc


def dmaap(t, offset, dims):
    """AP over dram tensor handle t with element offset and [step,count] dims."""
    base = t[tuple(slice(None) for _ in t.shape)]
    return bass.AP(tensor=base.tensor, offset=offset, ap=[list(d) for d in dims])


def build_body(nc, tc, x_in, out_fc, w_params, wfcT, gb, sel_in,
               a_act, convout, pooled, h_bin, cc_in, cc_out, st_dram):
    import contextlib
    ctx = contextlib.ExitStack()
    singles = ctx.enter_context(tc.tile_pool(name="singles", bufs=1))
    stats_pool = ctx.enter_context(tc.tile_pool(name="stats", bufs=1))
    small = ctx.enter_context(tc.tile_pool(name="small", bufs=4))

    # ---- load all weights once ----
    w_sb = []
    for li in range(6):
        cin, cout, kh, kw, hin, ps, sg, ch, oh = conv_dims(li)
        tiles = []
        for si, (k0, nkh) in enumerate(kh_splits(cin, kh)):
            k = sg * cin * nkh
            wt = singles.tile([k, kw, sg * cout], BF16, tag=f"w{li}_{si}")
            nc.sync.dma_start(out=wt, in_=w_params[li][si][:, :, :])
            tiles.append(wt)
        w_sb.append(tiles)
    wfc_sb = singles.tile([128, 2, 50], BF16, tag="wfc")
    nc.sync.dma_start(out=wfc_sb, in_=dmaap(wfcT, 0, [[50, 128], [6400, 2], [1, 50]]))
    sel_sb = singles.tile([128, 219], F32, tag="sel")
    nc.sync.dma_start(out=sel_sb, in_=sel_in[:, :])
    gb_sb = [singles.tile([c, 2], F32, tag=f"gb{i}", name=f"gb_sb{i}")
             for i, c in zip(range(7), [3, 8, 16, 32, 32, 64, 64])]
    for i in range(7):
        nc.sync.dma_start(out=gb_sb[i], in_=gb[i][:, :])
    eps_sb = singles.tile([64, 1], F32, tag="eps")
    nc.vector.memset(eps_sb, EPS)

    # HAM ignition: only K=128 matmuls un-throttle the PE clock
    # (1.2 -> 2.4 GHz); once warm, small-K conv matmuls sustain it. Fire a
    # ~5us K=128 burst at the start of each conv phase (the PE re-throttles
    # during the DVE-only pool/stats passes between layers).
    w_warm = singles.tile([128, 256], BF16, tag="w_warm")
    nc.vector.memset(w_warm, 1.0)
    heat_pool = ctx.enter_context(tc.tile_pool(name="heat", bufs=1, space="PSUM"))
    heat_ps = heat_pool.tile([64, 256], F32, tag="heat")

    def ignite(n=24):
        for _ in range(n):
            nc.tensor.matmul(heat_ps, w_warm[:, 0:64], w_warm,
                             start=True, stop=True)

    sel_off = [0, 3, 11, 27, 59, 91, 155]  # column offsets per layer in sel_sb
    sel_w = [3, 8, 16, 32, 32, 64, 64]

    def stats_finish(li, acc, n_tot):
        """acc: [128, 2] sbuf partials -> allreduce -> s,t in st_dram[li].
        li: 0..6 (bn index). Returns nothing; st_dram[li] written."""
        c = sel_w[li]
        with tc.tile_pool(name=f"stp{li}", bufs=1, space="PSUM") as psp:
            ps = psp.tile([c, 2], F32)
            nc.tensor.matmul(ps, sel_sb[:, sel_off[li]:sel_off[li] + c], acc,
                             start=True, stop=True)
            red = small.tile([c, 2], F32, tag="red")
            nc.vector.tensor_copy(red, ps)
        nc.sync.dma_start(out=cc_in[li][:, :], in_=red)
        nc.gpsimd.collective_compute(
            "AllReduce", mybir.AluOpType.add,
            replica_groups=[list(range(N_CORES))],
            ins=[cc_in[li][:, :]], outs=[cc_out[li][:, :]])
        tot = small.tile([c, 2], F32, tag="tot")
        nc.sync.dma_start(out=tot, in_=cc_out[li][:, :])
        # mean/ex2
        mv = small.tile([c, 2], F32, tag="mv")
        nc.scalar.activation(mv, tot, mybir.ActivationFunctionType.Copy,
                             bias=0.0, scale=1.0 / n_tot)
        m2 = small.tile([c, 1], F32, tag="m2")
        nc.scalar.square(m2, mv[:, 0:1])
        var = small.tile([c, 1], F32, tag="var")
        nc.vector.tensor_sub(var, mv[:, 1:2], m2)
        sd = small.tile([c, 1], F32, tag="sd")
        nc.scalar.activation(sd, var, mybir.ActivationFunctionType.Sqrt,
                             bias=eps_sb[0:c, :], scale=1.0)
        rstd = small.tile([c, 1], F32, tag="rstd")
        nc.vector.reciprocal(rstd, sd)
        st = small.tile([c, 2], F32, tag="st")
        nc.vector.tensor_mul(st[:, 0:1], gb_sb[li][:, 0:1], rstd)       # s
        ms = small.tile([c, 1], F32, tag="ms")
        nc.vector.tensor_mul(ms, mv[:, 0:1], st[:, 0:1])
        nc.vector.tensor_sub(st[:, 1:2], gb_sb[li][:, 1:2], ms)         # t
        nc.sync.dma_start(out=st_dram[li][:, :], in_=st)

    # ================= bn1 over x, binarize -> act0 =================
    # partition p = c*32 + n*4 + yc  (c:3, n:8, yc:4 chunks of 56 rows),
    # free chunked in 4 pieces of 14 rows to bound SBUF usage.
    XF = 14 * 224
    acc0 = stats_pool.tile([128, 2], F32, tag="acc0")
    nc.vector.memset(acc0, 0.0)
    with tc.tile_pool(name="bn1", bufs=2) as bn1p:
        for ci in range(4):
            xt = bn1p.tile([96, XF], F32, tag="xt")
            off = ci * XF
            nc.sync.dma_start(out=xt, in_=dmaap(
                x_in, off, [[4 * XF, 96], [1, XF]]))
            r0 = bn1p.tile([96, 1], F32, tag="r0")
            nc.vector.tensor_reduce(out=r0, in_=xt,
                                    axis=mybir.AxisListType.X,
                                    op=mybir.AluOpType.add)
            nc.vector.tensor_add(acc0[0:96, 0:1], acc0[0:96, 0:1], r0)
            sq0 = bn1p.tile([96, XF], BF16, tag="sq0")
            r1 = bn1p.tile([96, 1], F32, tag="r1")
            nc.scalar.activation(sq0, xt, mybir.ActivationFunctionType.Square,
                                 accum_out=r1)
            nc.vector.tensor_add(acc0[0:96, 1:2], acc0[0:96, 1:2], r1)
        stats_finish(0, acc0, 64 * 224 * 224)
        # broadcast s,t -> [96, 2] and sign
        st0 = bn1p.tile([96, 2], F32, tag="st0")
        st0b = st0[:, :]
        for yc in range(4):
            # partitions p = n*12 + c*4 + yc for (n, c); partition pitch = 2
            dst = bass.AP(tensor=st0b.tensor, offset=st0b.offset + yc * st0b.ap[0][0],
                          ap=[[4 * st0b.ap[0][0], 24], [1, 2]])
            nc.sync.dma_start(out=dst,
                              in_=dmaap(st_dram[0], 0, [[0, 8], [2, 3], [1, 2]]))
        for ci in range(4):
            xt = bn1p.tile([96, XF], F32, tag="xt")
            off = ci * XF
            nc.sync.dma_start(out=xt, in_=dmaap(
                x_in, off, [[4 * XF, 96], [1, XF]]))
            bin0 = bn1p.tile([96, XF], BF16, tag="bin0")
            nc.scalar.activation(bin0, xt, mybir.ActivationFunctionType.Sign,
                                 bias=st0[:, 1:2], scale=st0[:, 0:1])
            nc.sync.dma_start(
                out=dmaap(a_act[0], off, [[4 * XF, 96], [1, XF]]),
                in_=bin0)

    # ================= conv layers =================
    for li in range(6):
        cin, cout, kh, kw, hin, ps, sg, ch, oh = conv_dims(li)
        cw = ch
        win = hin
        splits = kh_splits(cin, kh)
        m = sg * cout
        ry = psum_rows(cw)
        yb = 16 if ch > 16 else ch
        yb = (yb // ry) * ry if ch > 16 else ch

        with tc.tile_pool(name=f"c{li}R", bufs=3) as rpool, \
             tc.tile_pool(name=f"c{li}P", bufs=6, space="PSUM") as pspool, \
             tc.tile_pool(name=f"c{li}S", bufs=3) as stpool:
            ignite()
            for g in range(NS // sg):
                n0 = g * sg
                y0 = 0
                while y0 < ch:
                    nb_rows = min(yb, ch - y0)
                    # load R tiles (kh-replicated input rows)
                    rts = []
                    for si, (k0, nkh) in enumerate(splits):
                        kdim = sg * cin * nkh
                        rt = rpool.tile([kdim, nb_rows, win], BF16,
                                        tag=f"R{si}")
                        src = dmaap(
                            a_act[li],
                            n0 * cin * win * win + (y0 + k0) * win,
                            [[win * win, sg * cin],
                             [win, nkh], [1, nb_rows * win]])
                        nc.sync.dma_start(out=rt, in_=src)
                        rts.append(rt)
                    ry0 = 0
                    while ry0 < nb_rows:
                        nr = min(ry, nb_rows - ry0)
                        pst = pspool.tile([m, ry, cw], F32, tag="ps")
                        first = True
                        for si, (k0, nkh) in enumerate(splits):
                            for kx in range(kw):
                                nc.tensor.matmul(
                                    pst[:, 0:nr, :],
                                    w_sb[li][si][:, kx, :],
                                    rts[si][:, ry0:ry0 + nr, kx:kx + cw],
                                    start=first,
                                    stop=(si == len(splits) - 1 and kx == kw - 1))
                                first = False
                        stg = stpool.tile([m, ry, cw], F16, tag="stg")
                        if (ry0 // ry) % 2 == 0:
                            nc.scalar.copy(stg[:, 0:nr, :], pst[:, 0:nr, :])
                        else:
                            nc.vector.tensor_copy(stg[:, 0:nr, :], pst[:, 0:nr, :])
                        # store to convout dram
                        dst = dmaap(
                            convout[li],
                            n0 * cout * ch * cw + (y0 + ry0) * cw,
                            [[ch * cw, sg * cout], [1, nr * cw]])
                        nc.sync.dma_start(out=dst, in_=stg[:, 0:nr, :])
                        ry0 += nr
                    y0 += nb_rows

        # ---- pool + stats pass ----
        nb = min(NS, 128 // cout)      # samples per pass
        p_used = cout * nb
        ox = oh
        acc = stats_pool.tile([128, 2], F32, tag=f"acc{li + 1}")
        nc.vector.memset(acc, 0.0)
        # chunk pooled rows
        oyc = oh
        while (ps * (oyc - 1) + 3) * cw * 2 > 12000:
            oyc = (oyc + 1) // 2
        max_rows = ps * (oyc - 1) + 3
        with tc.tile_pool(name=f"p{li}", bufs=2) as ppool:
            for pa in range(NS // nb):
                npa = pa * nb
                oy0 = 0
                while oy0 < oh:
                    noy = min(oyc, oh - oy0)
                    rows = ps * (noy - 1) + 3
                    ld = ppool.tile([p_used, max_rows, cw], F16, tag="ld")
                    src = dmaap(
                        convout[li],
                        npa * cout * ch * cw + (oy0 * ps) * cw,
                        [[ch * cw, nb * cout], [1, rows * cw]])
                    nc.sync.dma_start(out=ld[:, 0:rows, :], in_=src)
                    ldb = ld[:, :, :]

                    def ldv(dx):
                        return bass.AP(
                            tensor=ldb.tensor, offset=ldb.offset + dx,
                            ap=[[ldb.ap[0][0], p_used], [cw, rows], [ps, ox]])

                    ph = ppool.tile([p_used, max_rows, ox], F16, tag="ph")
                    nc.vector.tensor_max(ph[:, 0:rows, :], ldv(0), ldv(1))
                    nc.vector.tensor_max(ph[:, 0:rows, :], ph[:, 0:rows, :], ldv(2))
                    phb = ph[:, :, :]

                    def phv(dy):
                        return bass.AP(
                            tensor=phb.tensor, offset=phb.offset + dy * ox,
                            ap=[[phb.ap[0][0], p_used], [ps * ox, noy], [1, ox]])

                    pv = ppool.tile([p_used, oyc, ox], F16, tag="pv")
                    nc.vector.tensor_max(pv[:, 0:noy, :], phv(0), phv(1))
                    nc.vector.tensor_max(pv[:, 0:noy, :], pv[:, 0:noy, :], phv(2))
                    # stats
                    red = ppool.tile([p_used, 1], F32, tag="redp")
                    nc.vector.tensor_reduce(out=red, in_=pv[:, 0:noy, :],
                                            axis=mybir.AxisListType.XY,
                                            op=mybir.AluOpType.add)
                    nc.vector.tensor_add(acc[0:p_used, 0:1], acc[0:p_used, 0:1], red)
                    sqs = ppool.tile([p_used, oyc, ox], BF16, tag="sqs")
                    red2 = ppool.tile([p_used, 1], F32, tag="redq")
                    nc.scalar.activation(sqs[:, 0:noy, :], pv[:, 0:noy, :],
                                         mybir.ActivationFunctionType.Square,
                                         accum_out=red2)
                    nc.vector.tensor_add(acc[0:p_used, 1:2], acc[0:p_used, 1:2], red2)
                    # store pooled
                    dst = dmaap(
                        pooled[li],
                        npa * cout * oh * ox + oy0 * ox,
                        [[oh * ox, nb * cout], [1, noy * ox]])
                    nc.sync.dma_start(out=dst, in_=pv[:, 0:noy, :])
                    oy0 += noy

        stats_finish(li + 1, acc, 64 * oh * oh)

        # ---- binarize pass: pooled -> a_act[li+1] (or h_bin for last) ----
        dst_t = a_act[li + 1] if li < 5 else h_bin
        stb = stats_pool.tile([128, 2], F32, tag=f"stb{li + 1}")
        nc.sync.dma_start(out=stb[0:p_used, :],
                          in_=dmaap(st_dram[li + 1], 0, [[0, nb], [2, cout], [1, 2]]))
        f_all = oh * oh
        with tc.tile_pool(name=f"b{li}", bufs=3) as bpool:
            for pa in range(NS // nb):
                npa = pa * nb
                f0 = 0
                fchunk = min(f_all, 4096)
                while f0 < f_all:
                    nf = min(fchunk, f_all - f0)
                    ld = bpool.tile([p_used, fchunk], F16, tag="bl")
                    nc.sync.dma_start(
                        out=ld[:, 0:nf],
                        in_=dmaap(pooled[li], npa * cout * f_all + f0,
                                  [[f_all, nb * cout], [1, nf]]))
                    bo = bpool.tile([p_used, fchunk], BF16, tag="bo")
                    nc.scalar.activation(bo[:, 0:nf], ld[:, 0:nf],
                                         mybir.ActivationFunctionType.Sign,
                                         bias=stb[0:p_used, 1:2],
                                         scale=stb[0:p_used, 0:1])
                    nc.sync.dma_start(
                        out=dmaap(dst_t, npa * cout * f_all + f0,
                                  [[f_all, nb * cout], [1, nf]]),
                        in_=bo[:, 0:nf])
                    f0 += nf

    # ================= FC =================
    with tc.tile_pool(name="fc", bufs=1) as fcp, \
         tc.tile_pool(name="fcps", bufs=1, space="PSUM") as fcps:
        ht = [fcp.tile([128, NS], BF16, tag=f"ht{i}", name=f"ht{i}") for i in range(2)]
        for i in range(2):
            nc.sync.dma_start(out=ht[i], in_=dmaap(
                h_bin, i * 128, [[1, 128], [256, NS]]))
        ps = fcps.tile([NS, 50], F32)
        nc.tensor.matmul(ps, ht[0], wfc_sb[:, 0, :], start=True, stop=False)
        nc.tensor.matmul(ps, ht[1], wfc_sb[:, 1, :], start=False, stop=True)
        fo = fcp.tile([NS, 50], F32, tag="fo")
        nc.vector.tensor_copy(fo, ps)
        nc.sync.dma_start(out=out_fc[:, :], in_=fo)

    ctx.close()


# ---------------------------------------------------------------------------
# entry point
# ---------------------------------------------------------------------------

_cache = {}


def _get_nc():
    if "nc" not in _cache:
        nc = bacc.Bacc("TRN2", target_bir_lowering=False, num_devices=N_CORES)
        _cache["nc"] = build(nc)
    return _cache["nc"]


def _shared_maps(inputs):
    shared = {}
    for li in range(6):
        w = np.asarray(inputs[f"w{li + 1}"], np.float32)
        cin, cout, kh, kw, hin, ps, sg, ch, oh = conv_dims(li)
        for si, arr in enumerate(pack_conv_weights(w, sg)):
            shared[f"w{li + 1}_{si}"] = arr
    shared["wfcT"] = ml_bf16(np.sign(np.asarray(inputs["wfc"], np.float32)).T.copy())
    for i, c in zip(range(1, 8), [3, 8, 16, 32, 32, 64, 64]):
        shared[f"gb{i}"] = np.stack([np.asarray(inputs[f"g{i}"], np.float32),
                                     np.asarray(inputs[f"b{i}"], np.float32)],
                                    axis=1).copy()
    sels = []
    for c in [3, 8, 16, 32, 32, 64, 64]:
        if c == 3:
            sel = np.zeros((128, 3), np.float32)
            for ch_ in range(3):
                for n in range(NS):
                    for yc in range(4):
                        sel[n * 12 + ch_ * 4 + yc, ch_] = 1.0
        else:
            sel = make_selector(c, min(NS, 128 // c))
        sels.append(sel)
    shared["sels"] = np.concatenate(sels, axis=1).copy()
    return shared


def _in_maps(inputs):
    x = np.asarray(inputs["x"], np.float32)
    if "shared" not in _cache:
        _cache["shared"] = _shared_maps(inputs)
    shared = _cache["shared"]
    in_maps = []
    for c in range(N_CORES):
        m = dict(shared)
        m["x"] = np.ascontiguousarray(x[c * NS:(c + 1) * NS])
        in_maps.append(m)
    return in_maps


def _make_runner(nc):
    """Cached jitted SPMD executor (run_bass_via_pjrt retraces per call)."""
    import jax
    from jax.experimental.shard_map import shard_map
    from jax.sharding import Mesh, PartitionSpec
    from concourse import bass2jax, mybir as _mb

    bass2jax.install_neuronx_cc_hook()
    partition_name = (nc.partition_id_tensor.name
                      if nc.partition_id_tensor else None)
    in_names, out_names, out_avals, zero_outs = [], [], [], []
    for alloc in nc.m.functions[0].allocations:
        if not isinstance(alloc, _mb.MemoryLocationSet):
            continue
        name = alloc.memorylocations[0].name
        if alloc.kind == "ExternalInput":
            if name != partition_name:
                in_names.append(name)
        elif alloc.kind == "ExternalOutput":
            out_names.append(name)
            shape = tuple(alloc.tensor_shape)
            dtype = _mb.dt.np(alloc.dtype)
            out_avals.append(jax.core.ShapedArray(shape, dtype))
            zero_outs.append(np.zeros((N_CORES * shape[0],) + shape[1:], dtype))
    n_params = len(in_names)
    all_names = in_names + out_names
    if partition_name is not None:
        all_names = all_names + [partition_name]
    donate = tuple(range(n_params, n_params + len(out_names)))

    def _body(*args):
        operands = list(args)
        if partition_name is not None:
            operands.append(bass2jax.partition_id_tensor())
        outs = bass2jax._bass_exec_p.bind(
            *operands, out_avals=tuple(out_avals), in_names=tuple(all_names),
            out_names=tuple(out_names), lowering_input_output_aliases=(),
            sim_require_finite=True, sim_require_nnan=True, nc=nc)
        return tuple(outs)

    devices = jax.devices()[:N_CORES]
    mesh = Mesh(np.asarray(devices), ("core",))
    nin = n_params + len(out_names)
    sharded = jax.jit(
        shard_map(_body, mesh=mesh, in_specs=(PartitionSpec("core"),) * nin,
                  out_specs=(PartitionSpec("core"),) * len(out_names),
                  check_rep=False),
        donate_argnums=donate, keep_unused=True)

    def run(in_maps):
        concat_in = [np.concatenate([np.asarray(in_maps[c][n])
                                     for c in range(N_CORES)], axis=0)
                     for n in in_names]
        out_arrs = sharded(*concat_in, *zero_outs)
        return np.asarray(out_arrs[out_names.index("out")])

    return run


def kernel(**inputs):
    nc = _get_nc()
    if "runner" not in _cache:
        _cache["runner"] = _make_runner(nc)
    return _cache["runner"](_in_maps(inputs))


def profile(**inputs):
    """Run with NTFF tracing; returns (exec_time_ns, results) or None."""
    import types
    try:
        from antenv import axon_hooks
    except ImportError:
        import antenv
        axon_hooks = types.ModuleType("antenv.axon_hooks")
        axon_hooks._hook = None
        def _set(h):
            axon_hooks._hook = h
        def _get():
            return axon_hooks._hook
        axon_hooks.set_axon_ntff_profile_hook = _set
        axon_hooks.get_axon_ntff_profile_hook = _get
        sys.modules["antenv.axon_hooks"] = axon_hooks
        antenv.axon_hooks = axon_hooks
    if axon_hooks.get_axon_ntff_profile_hook() is None:
        sys.path.insert(0, "/root/.axon_site")
        from trn_agent_boot.trn_boot import _ntff_profile_via_ctypes
        hook = _ntff_profile_via_ctypes("/opt/axon/libaxon_pjrt.so")
        if hook is not None:
            axon_hooks.set_axon_ntff_profile_hook(hook)
    import concourse.bass_utils as bu
    bu.upload_artifacts = lambda tmpdir: "(skipped)"
    nc = _get_nc()
    res = run_bass_kernel_spmd(nc, _in_maps(inputs), list(range(N_CORES)),
                               trace=True)
    return res.exec_time_ns


if __name__ == "__main__":
    rng = np.random.default_rng(0)
    fake = {"x": rng.standard_normal((64, 3, 224, 224), dtype=np.float32)}
    for i, (ci, co, kh, kw) in zip(range(1, 7),
                                   [(3, 8, 11, 11), (8, 16, 7, 7), (16, 32, 5, 5),
                                    (32, 32, 5, 5), (32, 64, 5, 5), (64, 64, 3, 3)]):
        fake[f"w{i}"] = rng.standard_normal((co, ci, kh, kw), dtype=np.float32) * 0.1
    for i, c in zip(range(1, 8), [3, 8, 16, 32, 32, 64, 64]):
        fake[f"g{i}"] = np.ones(c, np.float32)
        fake[f"b{i}"] = np.zeros(c, np.float32)
    fake["wfc"] = rng.standard_normal((50, 256), dtype=np.float32) * 0.1
    out = kernel(**fake)
    print("out", out.shape, out[0, :5])


# revision 21
# speedup vs baseline: 1.3634x; 1.3593x over previous
"""Trainium2 Bass kernel for BinSpecCNN: 7x (BN -> sign) + 6 binary convs
with maxpool + final FC, data-parallel over 8 NeuronCores (8 samples each).

Strategy per core:
  - bn_i + sign fused as ACT Sign(scale*x+bias) passes; batch statistics
    (per-channel sum/sumsq) all-reduced across cores per layer.
  - convs: bf16 matmuls (inputs/weights are exactly +-1 in bf16), contraction
    dim = (sample-blockdiag x Cin x KH) on partitions, KW looped via
    free-dim-shifted rhs with PSUM accumulation. Input rows are KH-replicated
    into SBUF by DMA.
  - maxpool: two InstPool stages (horizontal window then vertical) on f16.
  - conv outputs (integer-valued, |v| <= 800) stored in f16 (exact).
"""

import sys

sys.path.insert(0, "/opt/trn_rl_repo")

import numpy as np

import concourse.bass as bass
import concourse.mybir as mybir
import concourse.tile as tile
from concourse import bacc
from concourse.bass_utils import run_bass_kernel_spmd

F32 = mybir.dt.float32
F16 = mybir.dt.float16
BF16 = mybir.dt.bfloat16

N_CORES = 8
NS = 8  # samples per core
EPS = 1e-5

# (Cin, Cout, KH, KW, Hin, pool_stride, sample_group)
CONVS = [
    (3, 8, 11, 11, 224, 1, 2),
    (8, 16, 7, 7, 212, 2, 2),
    (16, 32, 5, 5, 102, 2, 1),
    (32, 32, 5, 5, 48, 2, 1),
    (32, 64, 5, 5, 21, 2, 1),
    (64, 64, 3, 3, 8, 2, 1),
]


def conv_dims(li):
    cin, cout, kh, kw, hin, ps, sg = CONVS[li]
    ch = hin - kh + 1            # conv output height/width
    oh = (ch - 3) // ps + 1      # pooled output height/width
    return cin, cout, kh, kw, hin, ps, sg, ch, oh


def kh_splits(cin, kh):
    """Split KH taps so each split's (cin*nkh) <= 128 partitions."""
    max_kh = 128 // cin
    splits = []
    k0 = 0
    while k0 < kh:
        n = min(max_kh, kh - k0)
        splits.append((k0, n))
        k0 += n
    return splits


# rows-of-conv-output per psum tile (free dim = ry*cw <= 512)
def psum_rows(cw):
    return max(1, min(512 // cw, 8))


# ---------------------------------------------------------------------------
# host-side weight packing
# ---------------------------------------------------------------------------

def pack_conv_weights(w, sg):
    """w: [Cout, Cin, KH, KW] float32 -> per-khsplit lhsT arrays
    [K, KW, M] bf16 with K=(s, c, kh), M=(s, co) block-diagonal over s."""
    wb = np.sign(w).astype(np.float32)
    cout, cin, kh, kw = w.shape
    out = []
    for (k0, nkh) in kh_splits(cin, kh):
        m = sg * cout
        k = sg * cin * nkh
        arr = np.zeros((k, kw, m), np.float32)
        for s in range(sg):
            for c in range(cin):
                for kk in range(nkh):
                    p = (s * cin + c) * nkh + kk
                    # lhsT[p, kw, s*cout + co] = wb[co, c, k0+kk, kw]
                    arr[p, :, s * cout:(s + 1) * cout] = wb[:, c, k0 + kk, :].T
        out.append(ml_bf16(arr))
    return out


def ml_bf16(a):
    import ml_dtypes
    return np.asarray(a, dtype=np.float32).astype(ml_dtypes.bfloat16)


def ml_f16(a):
    return np.asarray(a, dtype=np.float16)


def make_selector(c, nb):
    """[128, c] selector: partition p=(g*c+ch) -> column ch."""
    sel = np.zeros((128, c), np.float32)
    for ch in range(c):
        for g in range(nb):
            sel[g * c + ch, ch] = 1.0
    return sel


# ---------------------------------------------------------------------------
# device kernel builder
# ---------------------------------------------------------------------------

def build(nc):
    x_in = nc.declare_dram_parameter("x", [NS, 3, 224, 224], F32, isOutput=False)
    out_fc = nc.declare_dram_parameter("out", [NS, 50], F32, isOutput=True)

    # weights / constants (identical on all cores)
    w_params = []   # per layer: list of lhsT dram handles
    for li in range(6):
        cin, cout, kh, kw, hin, ps, sg, ch, oh = conv_dims(li)
        splits = kh_splits(cin, kh)
        handles = []
        for si, (k0, nkh) in enumerate(splits):
            k = sg * cin * nkh
            handles.append(nc.declare_dram_parameter(
                f"w{li + 1}_{si}", [k, kw, sg * cout], BF16, isOutput=False))
        w_params.append(handles)
    wfcT = nc.declare_dram_parameter("wfcT", [256, 50], BF16, isOutput=False)

    gb = [nc.declare_dram_parameter(f"gb{i}", [c, 2], F32, isOutput=False)
          for i, c in zip(range(1, 8), [3, 8, 16, 32, 32, 64, 64])]
    sel_in = nc.declare_dram_parameter("sels", [128, 3 + 8 + 16 + 32 + 32 + 64 + 64],
                                       F32, isOutput=False)

    # internal DRAM
    a_act = []   # binarized activations bf16, input of conv li
    for li in range(6):
        cin, cout, kh, kw, hin, ps, sg, ch, oh = conv_dims(li)
        a_act.append(nc.dram_tensor(f"act{li}", [NS, cin, hin, hin], BF16))
    convout = []
    pooled = []
    for li in range(6):
        cin, cout, kh, kw, hin, ps, sg, ch, oh = conv_dims(li)
        convout.append(nc.dram_tensor(f"conv{li}", [NS, cout, ch, ch], F16))
        pooled.append(nc.dram_tensor(f"pool{li}", [NS, cout, oh, oh], F16))
    h_bin = nc.dram_tensor("h_bin", [NS, 64, 2, 2], BF16)  # final binarized

    cc_in = [nc.dram_tensor(f"cc_in{i}", [c, 2], F32)
             for i, c in zip(range(7), [3, 8, 16, 32, 32, 64, 64])]
    cc_out = [nc.dram_tensor(f"cc_out{i}", [c, 2], F32, addr_space="Shared")
              for i, c in zip(range(7), [3, 8, 16, 32, 32, 64, 64])]
    st_dram = [nc.dram_tensor(f"st{i}", [c, 2], F32)
               for i, c in zip(range(7), [3, 8, 16, 32, 32, 64, 64])]

    with tile.TileContext(nc) as tc:
        build_body(nc, tc, x_in, out_fc, w_params, wfcT, gb, sel_in,
                   a_act, convout, pooled, h_bin, cc_in, cc_out, st_dram)
    nc.finalize()
    return nc


def dmaap(t, offset, dims):
    """AP over dram tensor handle t with element offset and [step,count] dims."""
    base = t[tuple(slice(None) for _ in t.shape)]
    return bass.AP(tensor=base.tensor, offset=offset, ap=[list(d) for d in dims])


def build_body(nc, tc, x_in, out_fc, w_params, wfcT, gb, sel_in,
               a_act, convout, pooled, h_bin, cc_in, cc_out, st_dram):
    import contextlib
    ctx = contextlib.ExitStack()
    singles = ctx.enter_context(tc.tile_pool(name="singles", bufs=1))
    stats_pool = ctx.enter_context(tc.tile_pool(name="stats", bufs=1))
    small = ctx.enter_context(tc.tile_pool(name="small", bufs=4))

    # ---- load all weights once ----
    w_sb = []
    for li in range(6):
        cin, cout, kh, kw, hin, ps, sg, ch, oh = conv_dims(li)
        tiles = []
        for si, (k0, nkh) in enumerate(kh_splits(cin, kh)):
            k = sg * cin * nkh
            wt = singles.tile([k, kw, sg * cout], BF16, tag=f"w{li}_{si}")
            nc.sync.dma_start(out=wt, in_=w_params[li][si][:, :, :])
            tiles.append(wt)
        w_sb.append(tiles)
    wfc_sb = singles.tile([128, 2, 50], BF16, tag="wfc")
    nc.sync.dma_start(out=wfc_sb, in_=dmaap(wfcT, 0, [[50, 128], [6400, 2], [1, 50]]))
    sel_sb = singles.tile([128, 219], F32, tag="sel")
    nc.sync.dma_start(out=sel_sb, in_=sel_in[:, :])
    gb_sb = [singles.tile([c, 2], F32, tag=f"gb{i}", name=f"gb_sb{i}")
             for i, c in zip(range(7), [3, 8, 16, 32, 32, 64, 64])]
    for i in range(7):
        nc.sync.dma_start(out=gb_sb[i], in_=gb[i][:, :])
    eps_sb = singles.tile([64, 1], F32, tag="eps")
    nc.vector.memset(eps_sb, EPS)

    # HAM ignition: only K=128 matmuls un-throttle the PE clock
    # (1.2 -> 2.4 GHz); once warm, small-K conv matmuls sustain it. Fire a
    # ~5us K=128 burst at the start of each conv phase (the PE re-throttles
    # during the DVE-only pool/stats passes between layers).
    w_warm = singles.tile([128, 256], BF16, tag="w_warm")
    nc.vector.memset(w_warm, 1.0)
    heat_pool = ctx.enter_context(tc.tile_pool(name="heat", bufs=1, space="PSUM"))
    heat_ps = heat_pool.tile([64, 256], F32, tag="heat")

    def ignite(n=24):
        for _ in range(n):
            nc.tensor.matmul(heat_ps, w_warm[:, 0:64], w_warm,
                             start=True, stop=True)

    sel_off = [0, 3, 11, 27, 59, 91, 155]  # column offsets per layer in sel_sb
    sel_w = [3, 8, 16, 32, 32, 64, 64]

    def stats_finish(li, acc, n_tot):
        """acc: [128, 2] sbuf partials -> allreduce -> s,t in st_dram[li].
        li: 0..6 (bn index). Returns nothing; st_dram[li] written."""
        c = sel_w[li]
        with tc.tile_pool(name=f"stp{li}", bufs=1, space="PSUM") as psp:
            ps = psp.tile([c, 2], F32)
            nc.tensor.matmul(ps, sel_sb[:, sel_off[li]:sel_off[li] + c], acc,
                             start=True, stop=True)
            red = small.tile([c, 2], F32, tag="red")
            nc.vector.tensor_copy(red, ps)
        nc.sync.dma_start(out=cc_in[li][:, :], in_=red)
        nc.gpsimd.collective_compute(
            "AllReduce", mybir.AluOpType.add,
            replica_groups=[list(range(N_CORES))],
            ins=[cc_in[li][:, :]], outs=[cc_out[li][:, :]])
        tot = small.tile([c, 2], F32, tag="tot")
        nc.sync.dma_start(out=tot, in_=cc_out[li][:, :])
        # mean/ex2
        mv = small.tile([c, 2], F32, tag="mv")
        nc.scalar.activation(mv, tot, mybir.ActivationFunctionType.Copy,
                             bias=0.0, scale=1.0 / n_tot)
        m2 = small.tile([c, 1], F32, tag="m2")
        nc.scalar.square(m2, mv[:, 0:1])
        var = small.tile([c, 1], F32, tag="var")
        nc.vector.tensor_sub(var, mv[:, 1:2], m2)
        sd = small.tile([c, 1], F32, tag="sd")
        nc.scalar.activation(sd, var, mybir.ActivationFunctionType.Sqrt,
                             bias=eps_sb[0:c, :], scale=1.0)
        rstd = small.tile([c, 1], F32, tag="rstd")
        nc.vector.reciprocal(rstd, sd)
        st = small.tile([c, 2], F32, tag="st")
        nc.vector.tensor_mul(st[:, 0:1], gb_sb[li][:, 0:1], rstd)       # s
        ms = small.tile([c, 1], F32, tag="ms")
        nc.vector.tensor_mul(ms, mv[:, 0:1], st[:, 0:1])
        nc.vector.tensor_sub(st[:, 1:2], gb_sb[li][:, 1:2], ms)         # t
        nc.sync.dma_start(out=st_dram[li][:, :], in_=st)

    # ================= bn1 over x, binarize -> act0 =================
    # partition p = c*32 + n*4 + yc  (c:3, n:8, yc:4 chunks of 56 rows),
    # free chunked in 4 pieces of 14 rows to bound SBUF usage.
    XF = 14 * 224
    acc0 = stats_pool.tile([128, 2], F32, tag="acc0")
    nc.vector.memset(acc0, 0.0)
    with tc.tile_pool(name="bn1", bufs=2) as bn1p:
        for ci in range(4):
            xt = bn1p.tile([96, XF], F32, tag="xt")
            off = ci * XF
            nc.sync.dma_start(out=xt, in_=dmaap(
                x_in, off, [[4 * XF, 96], [1, XF]]))
            r0 = bn1p.tile([96, 1], F32, tag="r0")
            nc.vector.tensor_reduce(out=r0, in_=xt,
                                    axis=mybir.AxisListType.X,
                                    op=mybir.AluOpType.add)
            nc.vector.tensor_add(acc0[0:96, 0:1], acc0[0:96, 0:1], r0)
            sq0 = bn1p.tile([96, XF], BF16, tag="sq0")
            r1 = bn1p.tile([96, 1], F32, tag="r1")
            nc.scalar.activation(sq0, xt, mybir.ActivationFunctionType.Square,
                                 accum_out=r1)
            nc.vector.tensor_add(acc0[0:96, 1:2], acc0[0:96, 1:2], r1)
        stats_finish(0, acc0, 64 * 224 * 224)
        # broadcast s,t -> [96, 2] and sign
        st0 = bn1p.tile([96, 2], F32, tag="st0")
        st0b = st0[:, :]
        for yc in range(4):
            # partitions p = n*12 + c*4 + yc for (n, c); partition pitch = 2
            dst = bass.AP(tensor=st0b.tensor, offset=st0b.offset + yc * st0b.ap[0][0],
                          ap=[[4 * st0b.ap[0][0], 24], [1, 2]])
            nc.sync.dma_start(out=dst,
                              in_=dmaap(st_dram[0], 0, [[0, 8], [2, 3], [1, 2]]))
        for ci in range(4):
            xt = bn1p.tile([96, XF], F32, tag="xt")
            off = ci * XF
            nc.sync.dma_start(out=xt, in_=dmaap(
                x_in, off, [[4 * XF, 96], [1, XF]]))
            bin0 = bn1p.tile([96, XF], BF16, tag="bin0")
            nc.scalar.activation(bin0, xt, mybir.ActivationFunctionType.Sign,
                                 bias=st0[:, 1:2], scale=st0[:, 0:1])
            nc.sync.dma_start(
                out=dmaap(a_act[0], off, [[4 * XF, 96], [1, XF]]),
                in_=bin0)

    # ================= conv layers =================
    for li in range(6):
        cin, cout, kh, kw, hin, ps, sg, ch, oh = conv_dims(li)
        cw = ch
        win = hin
        splits = kh_splits(cin, kh)
        m = sg * cout
        ry = psum_rows(cw)
        yb = 16 if ch > 16 else ch
        yb = (yb // ry) * ry if ch > 16 else ch

        with tc.tile_pool(name=f"c{li}R", bufs=3) as rpool, \
             tc.tile_pool(name=f"c{li}P", bufs=6, space="PSUM") as pspool, \
             tc.tile_pool(name=f"c{li}S", bufs=3) as stpool:
            ignited = False
            for g in range(NS // sg):
                n0 = g * sg
                y0 = 0
                while y0 < ch:
                    nb_rows = min(yb, ch - y0)
                    # load R tiles (kh-replicated input rows)
                    rts = []
                    for si, (k0, nkh) in enumerate(splits):
                        kdim = sg * cin * nkh
                        rt = rpool.tile([kdim, nb_rows, win], BF16,
                                        tag=f"R{si}")
                        src = dmaap(
                            a_act[li],
                            n0 * cin * win * win + (y0 + k0) * win,
                            [[win * win, sg * cin],
                             [win, nkh], [1, nb_rows * win]])
                        nc.sync.dma_start(out=rt, in_=src)
                        rts.append(rt)
                    if not ignited:
                        # dep-stitch: ignition must fire when conv data is
                        # ready, not at phase start (warmth expires in ~3.4us)
                        nc.vector.tensor_copy(w_warm[0:1, 0:1],
                                              rts[0][0:1, 0:1, 0:1])
                        ignite()
                        ignited = True
                    ry0 = 0
                    while ry0 < nb_rows:
                        nr = min(ry, nb_rows - ry0)
                        pst = pspool.tile([m, ry, cw], F32, tag="ps")
                        first = True
                        for si, (k0, nkh) in enumerate(splits):
                            for kx in range(kw):
                                nc.tensor.matmul(
                                    pst[:, 0:nr, :],
                                    w_sb[li][si][:, kx, :],
                                    rts[si][:, ry0:ry0 + nr, kx:kx + cw],
                                    start=first,
                                    stop=(si == len(splits) - 1 and kx == kw - 1))
                                first = False
                        stg = stpool.tile([m, ry, cw], F16, tag="stg")
                        if (ry0 // ry) % 2 == 0:
                            nc.scalar.copy(stg[:, 0:nr, :], pst[:, 0:nr, :])
                        else:
                            nc.vector.tensor_copy(stg[:, 0:nr, :], pst[:, 0:nr, :])
                        # store to convout dram
                        dst = dmaap(
                            convout[li],
                            n0 * cout * ch * cw + (y0 + ry0) * cw,
                            [[ch * cw, sg * cout], [1, nr * cw]])
                        nc.sync.dma_start(out=dst, in_=stg[:, 0:nr, :])
                        ry0 += nr
                    y0 += nb_rows

        # ---- pool + stats pass ----
        nb = min(NS, 128 // cout)      # samples per pass
        p_used = cout * nb
        ox = oh
        acc = stats_pool.tile([128, 2], F32, tag=f"acc{li + 1}")
        nc.vector.memset(acc, 0.0)
        # chunk pooled rows
        oyc = oh
        while (ps * (oyc - 1) + 3) * cw * 2 > 12000:
            oyc = (oyc + 1) // 2
        max_rows = ps * (oyc - 1) + 3
        with tc.tile_pool(name=f"p{li}", bufs=2) as ppool:
            for pa in range(NS // nb):
                npa = pa * nb
                oy0 = 0
                while oy0 < oh:
                    noy = min(oyc, oh - oy0)
                    rows = ps * (noy - 1) + 3
                    ld = ppool.tile([p_used, max_rows, cw], F16, tag="ld")
                    src = dmaap(
                        convout[li],
                        npa * cout * ch * cw + (oy0 * ps) * cw,
                        [[ch * cw, nb * cout], [1, rows * cw]])
                    nc.sync.dma_start(out=ld[:, 0:rows, :], in_=src)
                    ldb = ld[:, :, :]

                    def ldv(dx):
                        return bass.AP(
                            tensor=ldb.tensor, offset=ldb.offset + dx,
                            ap=[[ldb.ap[0][0], p_used], [cw, rows], [ps, ox]])

                    ph = ppool.tile([p_used, max_rows, ox], F16, tag="ph")
                    nc.vector.tensor_max(ph[:, 0:rows, :], ldv(0), ldv(1))
                    nc.vector.tensor_max(ph[:, 0:rows, :], ph[:, 0:rows, :], ldv(2))
                    phb = ph[:, :, :]

                    def phv(dy):
                        return bass.AP(
                            tensor=phb.tensor, offset=phb.offset + dy * ox,
                            ap=[[phb.ap[0][0], p_used], [ps * ox, noy], [1, ox]])

                    pv = ppool.tile([p_used, oyc, ox], F16, tag="pv")
                    nc.vector.tensor_max(pv[:, 0:noy, :], phv(0), phv(1))
                    nc.vector.tensor_max(pv[:, 0:noy, :], pv[:, 0:noy, :], phv(2))
                    # stats
                    red = ppool.tile([p_used, 1], F32, tag="redp")
                    nc.vector.tensor_reduce(out=red, in_=pv[:, 0:noy, :],
                                            axis=mybir.AxisListType.XY,
                                            op=mybir.AluOpType.add)
                    nc.vector.tensor_add(acc[0:p_used, 0:1], acc[0:p_used, 0:1], red)
                    sqs = ppool.tile([p_used, oyc, ox], BF16, tag="sqs")
                    red2 = ppool.tile([p_used, 1], F32, tag="redq")
                    nc.scalar.activation(sqs[:, 0:noy, :], pv[:, 0:noy, :],
                                         mybir.ActivationFunctionType.Square,
                                         accum_out=red2)
                    nc.vector.tensor_add(acc[0:p_used, 1:2], acc[0:p_used, 1:2], red2)
                    # store pooled
                    dst = dmaap(
                        pooled[li],
                        npa * cout * oh * ox + oy0 * ox,
                        [[oh * ox, nb * cout], [1, noy * ox]])
                    nc.sync.dma_start(out=dst, in_=pv[:, 0:noy, :])
                    oy0 += noy

        stats_finish(li + 1, acc, 64 * oh * oh)

        # ---- binarize pass: pooled -> a_act[li+1] (or h_bin for last) ----
        dst_t = a_act[li + 1] if li < 5 else h_bin
        stb = stats_pool.tile([128, 2], F32, tag=f"stb{li + 1}")
        nc.sync.dma_start(out=stb[0:p_used, :],
                          in_=dmaap(st_dram[li + 1], 0, [[0, nb], [2, cout], [1, 2]]))
        f_all = oh * oh
        with tc.tile_pool(name=f"b{li}", bufs=3) as bpool:
            for pa in range(NS // nb):
                npa = pa * nb
                f0 = 0
                fchunk = min(f_all, 4096)
                while f0 < f_all:
                    nf = min(fchunk, f_all - f0)
                    ld = bpool.tile([p_used, fchunk], F16, tag="bl")
                    nc.sync.dma_start(
                        out=ld[:, 0:nf],
                        in_=dmaap(pooled[li], npa * cout * f_all + f0,
                                  [[f_all, nb * cout], [1, nf]]))
                    bo = bpool.tile([p_used, fchunk], BF16, tag="bo")
                    nc.scalar.activation(bo[:, 0:nf], ld[:, 0:nf],
                                         mybir.ActivationFunctionType.Sign,
                                         bias=stb[0:p_used, 1:2],
                                         scale=stb[0:p_used, 0:1])
                    nc.sync.dma_start(
                        out=dmaap(dst_t, npa * cout * f_all + f0,
                                  [[f_all, nb * cout], [1, nf]]),
                        in_=bo[:, 0:nf])
                    f0 += nf

    # ================= FC =================
    with tc.tile_pool(name="fc", bufs=1) as fcp, \
         tc.tile_pool(name="fcps", bufs=1, space="PSUM") as fcps:
        ht = [fcp.tile([128, NS], BF16, tag=f"ht{i}", name=f"ht{i}") for i in range(2)]
        for i in range(2):
            nc.sync.dma_start(out=ht[i], in_=dmaap(
                h_bin, i * 128, [[1, 128], [256, NS]]))
        ps = fcps.tile([NS, 50], F32)
        nc.tensor.matmul(ps, ht[0], wfc_sb[:, 0, :], start=True, stop=False)
        nc.tensor.matmul(ps, ht[1], wfc_sb[:, 1, :], start=False, stop=True)
        fo = fcp.tile([NS, 50], F32, tag="fo")
        nc.vector.tensor_copy(fo, ps)
        nc.sync.dma_start(out=out_fc[:, :], in_=fo)

    ctx.close()


# ---------------------------------------------------------------------------
# entry point
# ---------------------------------------------------------------------------

_cache = {}


def _get_nc():
    if "nc" not in _cache:
        nc = bacc.Bacc("TRN2", target_bir_lowering=False, num_devices=N_CORES)
        _cache["nc"] = build(nc)
    return _cache["nc"]


def _shared_maps(inputs):
    shared = {}
    for li in range(6):
        w = np.asarray(inputs[f"w{li + 1}"], np.float32)
        cin, cout, kh, kw, hin, ps, sg, ch, oh = conv_dims(li)
        for si, arr in enumerate(pack_conv_weights(w, sg)):
            shared[f"w{li + 1}_{si}"] = arr
    shared["wfcT"] = ml_bf16(np.sign(np.asarray(inputs["wfc"], np.float32)).T.copy())
    for i, c in zip(range(1, 8), [3, 8, 16, 32, 32, 64, 64]):
        shared[f"gb{i}"] = np.stack([np.asarray(inputs[f"g{i}"], np.float32),
                                     np.asarray(inputs[f"b{i}"], np.float32)],
                                    axis=1).copy()
    sels = []
    for c in [3, 8, 16, 32, 32, 64, 64]:
        if c == 3:
            sel = np.zeros((128, 3), np.float32)
            for ch_ in range(3):
                for n in range(NS):
                    for yc in range(4):
                        sel[n * 12 + ch_ * 4 + yc, ch_] = 1.0
        else:
            sel = make_selector(c, min(NS, 128 // c))
        sels.append(sel)
    shared["sels"] = np.concatenate(sels, axis=1).copy()
    return shared


def _in_maps(inputs):
    x = np.asarray(inputs["x"], np.float32)
    if "shared" not in _cache:
        _cache["shared"] = _shared_maps(inputs)
    shared = _cache["shared"]
    in_maps = []
    for c in range(N_CORES):
        m = dict(shared)
        m["x"] = np.ascontiguousarray(x[c * NS:(c + 1) * NS])
        in_maps.append(m)
    return in_maps


def _make_runner(nc):
    """Cached jitted SPMD executor (run_bass_via_pjrt retraces per call)."""
    import jax
    from jax.experimental.shard_map import shard_map
    from jax.sharding import Mesh, PartitionSpec
    from concourse import bass2jax, mybir as _mb

    bass2jax.install_neuronx_cc_hook()
    partition_name = (nc.partition_id_tensor.name
                      if nc.partition_id_tensor else None)
    in_names, out_names, out_avals, zero_outs = [], [], [], []
    for alloc in nc.m.functions[0].allocations:
        if not isinstance(alloc, _mb.MemoryLocationSet):
            continue
        name = alloc.memorylocations[0].name
        if alloc.kind == "ExternalInput":
            if name != partition_name:
                in_names.append(name)
        elif alloc.kind == "ExternalOutput":
            out_names.append(name)
            shape = tuple(alloc.tensor_shape)
            dtype = _mb.dt.np(alloc.dtype)
            out_avals.append(jax.core.ShapedArray(shape, dtype))
            zero_outs.append(np.zeros((N_CORES * shape[0],) + shape[1:], dtype))
    n_params = len(in_names)
    all_names = in_names + out_names
    if partition_name is not None:
        all_names = all_names + [partition_name]
    donate = tuple(range(n_params, n_params + len(out_names)))

    def _body(*args):
        operands = list(args)
        if partition_name is not None:
            operands.append(bass2jax.partition_id_tensor())
        outs = bass2jax._bass_exec_p.bind(
            *operands, out_avals=tuple(out_avals), in_names=tuple(all_names),
            out_names=tuple(out_names), lowering_input_output_aliases=(),
            sim_require_finite=True, sim_require_nnan=True, nc=nc)
        return tuple(outs)

    devices = jax.devices()[:N_CORES]
    mesh = Mesh(np.asarray(devices), ("core",))
    nin = n_params + len(out_names)
    sharded = jax.jit(
        shard_map(_body, mesh=mesh, in_specs=(PartitionSpec("core"),) * nin,
                  out_specs=(PartitionSpec("core"),) * len(out_names),
                  check_rep=False),
        donate_argnums=donate, keep_unused=True)

    def run(in_maps):
        concat_in = [np.concatenate([np.asarray(in_maps[c][n])
                                     for c in range(N_CORES)], axis=0)
                     for n in in_names]
        out_arrs = sharded(*concat_in, *zero_outs)
        return np.asarray(out_arrs[out_names.index("out")])

    return run


def kernel(**inputs):
    nc = _get_nc()
    if "runner" not in _cache:
        _cache["runner"] = _make_runner(nc)
    return _cache["runner"](_in_maps(inputs))


def profile(**inputs):
    """Run with NTFF tracing; returns (exec_time_ns, results) or None."""
    import types
    try:
        from antenv import axon_hooks
    except ImportError:
        import antenv
        axon_hooks = types.ModuleType("antenv.axon_hooks")
        axon_hooks._hook = None
        def _set(h):
            axon_hooks._hook = h
        def _get():
            return axon_hooks._hook
        axon_hooks.set_axon_ntff_profile_hook = _set
        axon_hooks.get_axon_ntff_profile_hook = _get
        sys.modules["antenv.axon_hooks"] = axon_hooks
        antenv.axon_hooks = axon_hooks
    if axon_hooks.get_axon_ntff_profile_hook() is None:
        sys.path.insert(0, "/root/.axon_site")
        from trn_agent_boot.trn_boot import _ntff_profile_via_ctypes
        hook = _ntff_profile_via_ctypes("/opt/axon/libaxon_pjrt.so")
        if hook is not None:
            axon_hooks.set_axon_ntff_profile_hook(hook)
    import concourse.bass_utils as bu
    bu.upload_artifacts = lambda tmpdir: "(skipped)"
    nc = _get_nc()
    res = run_bass_kernel_spmd(nc, _in_maps(inputs), list(range(N_CORES)),
                               trace=True)
    return res.exec_time_ns


if __name__ == "__main__":
    rng = np.random.default_rng(0)
    fake = {"x": rng.standard_normal((64, 3, 224, 224), dtype=np.float32)}
    for i, (ci, co, kh, kw) in zip(range(1, 7),
                                   [(3, 8, 11, 11), (8, 16, 7, 7), (16, 32, 5, 5),
                                    (32, 32, 5, 5), (32, 64, 5, 5), (64, 64, 3, 3)]):
        fake[f"w{i}"] = rng.standard_normal((co, ci, kh, kw), dtype=np.float32) * 0.1
    for i, c in zip(range(1, 8), [3, 8, 16, 32, 32, 64, 64]):
        fake[f"g{i}"] = np.ones(c, np.float32)
        fake[f"b{i}"] = np.zeros(c, np.float32)
    fake["wfc"] = rng.standard_normal((50, 256), dtype=np.float32) * 0.1
    out = kernel(**fake)
    print("out", out.shape, out[0, :5])


# revision 22
# speedup vs baseline: 1.8683x; 1.3703x over previous
"""Trainium2 Bass kernel for BinSpecCNN: 7x (BN -> sign) + 6 binary convs
with maxpool + final FC, data-parallel over 8 NeuronCores (8 samples each).

Strategy per core:
  - bn_i + sign fused as ACT Sign(scale*x+bias) passes; batch statistics
    (per-channel sum/sumsq) all-reduced across cores per layer.
  - convs: bf16 matmuls (inputs/weights are exactly +-1 in bf16), contraction
    dim = (sample-blockdiag x Cin x KH) on partitions, KW looped via
    free-dim-shifted rhs with PSUM accumulation. Input rows are KH-replicated
    into SBUF by DMA.
  - maxpool: two InstPool stages (horizontal window then vertical) on f16.
  - conv outputs (integer-valued, |v| <= 800) stored in f16 (exact).
"""

import sys

sys.path.insert(0, "/opt/trn_rl_repo")

import numpy as np

import concourse.bass as bass
import concourse.mybir as mybir
import concourse.tile as tile
from concourse import bacc
from concourse.bass_utils import run_bass_kernel_spmd

F32 = mybir.dt.float32
F16 = mybir.dt.float16
BF16 = mybir.dt.bfloat16

N_CORES = 8
NS = 8  # samples per core
EPS = 1e-5

# (Cin, Cout, KH, KW, Hin, pool_stride, sample_group)
CONVS = [
    (3, 8, 11, 11, 224, 1, 2),
    (8, 16, 7, 7, 212, 2, 2),
    (16, 32, 5, 5, 102, 2, 1),
    (32, 32, 5, 5, 48, 2, 1),
    (32, 64, 5, 5, 21, 2, 1),
    (64, 64, 3, 3, 8, 2, 1),
]


def conv_dims(li):
    cin, cout, kh, kw, hin, ps, sg = CONVS[li]
    ch = hin - kh + 1            # conv output height/width
    oh = (ch - 3) // ps + 1      # pooled output height/width
    return cin, cout, kh, kw, hin, ps, sg, ch, oh


def kh_splits(cin, kh):
    """Split KH taps so each split's (cin*nkh) <= 128 partitions."""
    max_kh = 128 // cin
    splits = []
    k0 = 0
    while k0 < kh:
        n = min(max_kh, kh - k0)
        splits.append((k0, n))
        k0 += n
    return splits


# rows-of-conv-output per psum tile (free dim = ry*cw <= 512)
def psum_rows(cw):
    return max(1, min(512 // cw, 8))


# ---------------------------------------------------------------------------
# host-side weight packing
# ---------------------------------------------------------------------------

def pack_conv_weights(w, sg):
    """w: [Cout, Cin, KH, KW] float32 -> per-khsplit lhsT arrays
    [K, KW, M] bf16 with K=(s, c, kh), M=(s, co) block-diagonal over s."""
    wb = np.sign(w).astype(np.float32)
    cout, cin, kh, kw = w.shape
    out = []
    for (k0, nkh) in kh_splits(cin, kh):
        m = sg * cout
        k = sg * cin * nkh
        arr = np.zeros((k, kw, m), np.float32)
        for s in range(sg):
            for c in range(cin):
                for kk in range(nkh):
                    p = (s * cin + c) * nkh + kk
                    # lhsT[p, kw, s*cout + co] = wb[co, c, k0+kk, kw]
                    arr[p, :, s * cout:(s + 1) * cout] = wb[:, c, k0 + kk, :].T
        out.append(ml_bf16(arr))
    return out


def ml_bf16(a):
    import ml_dtypes
    return np.asarray(a, dtype=np.float32).astype(ml_dtypes.bfloat16)


def ml_f16(a):
    return np.asarray(a, dtype=np.float16)


def make_selector(c, nb):
    """[128, c] selector: partition p=(g*c+ch) -> column ch."""
    sel = np.zeros((128, c), np.float32)
    for ch in range(c):
        for g in range(nb):
            sel[g * c + ch, ch] = 1.0
    return sel


# ---------------------------------------------------------------------------
# device kernel builder
# ---------------------------------------------------------------------------

def build(nc):
    x_in = nc.declare_dram_parameter("x", [NS, 3, 224, 224], F32, isOutput=False)
    out_fc = nc.declare_dram_parameter("out", [NS, 50], F32, isOutput=True)

    # weights / constants (identical on all cores)
    w_params = []   # per layer: list of lhsT dram handles
    for li in range(6):
        cin, cout, kh, kw, hin, ps, sg, ch, oh = conv_dims(li)
        splits = kh_splits(cin, kh)
        handles = []
        for si, (k0, nkh) in enumerate(splits):
            k = sg * cin * nkh
            handles.append(nc.declare_dram_parameter(
                f"w{li + 1}_{si}", [k, kw, sg * cout], BF16, isOutput=False))
        w_params.append(handles)
    wfcT = nc.declare_dram_parameter("wfcT", [256, 50], BF16, isOutput=False)

    gb = [nc.declare_dram_parameter(f"gb{i}", [c, 2], F32, isOutput=False)
          for i, c in zip(range(1, 8), [3, 8, 16, 32, 32, 64, 64])]
    sel_in = nc.declare_dram_parameter("sels", [128, 3 + 8 + 16 + 32 + 32 + 64 + 64],
                                       F32, isOutput=False)

    # internal DRAM
    a_act = []   # binarized activations bf16, input of conv li
    for li in range(6):
        cin, cout, kh, kw, hin, ps, sg, ch, oh = conv_dims(li)
        a_act.append(nc.dram_tensor(f"act{li}", [NS, cin, hin, hin], BF16))
    convout = []
    pooled = []
    for li in range(6):
        cin, cout, kh, kw, hin, ps, sg, ch, oh = conv_dims(li)
        convout.append(nc.dram_tensor(f"conv{li}", [NS, cout, ch, ch], F16))
        pooled.append(nc.dram_tensor(f"pool{li}", [NS, cout, oh, oh], F16))
    h_bin = nc.dram_tensor("h_bin", [NS, 64, 2, 2], BF16)  # final binarized

    cc_in = [nc.dram_tensor(f"cc_in{i}", [c, 2], F32)
             for i, c in zip(range(7), [3, 8, 16, 32, 32, 64, 64])]
    cc_out = [nc.dram_tensor(f"cc_out{i}", [c, 2], F32, addr_space="Shared")
              for i, c in zip(range(7), [3, 8, 16, 32, 32, 64, 64])]
    st_dram = [nc.dram_tensor(f"st{i}", [c, 2], F32)
               for i, c in zip(range(7), [3, 8, 16, 32, 32, 64, 64])]

    with tile.TileContext(nc) as tc:
        build_body(nc, tc, x_in, out_fc, w_params, wfcT, gb, sel_in,
                   a_act, convout, pooled, h_bin, cc_in, cc_out, st_dram)
    nc.finalize()
    return nc


def dmaap(t, offset, dims):
    """AP over dram tensor handle t with element offset and [step,count] dims."""
    base = t[tuple(slice(None) for _ in t.shape)]
    return bass.AP(tensor=base.tensor, offset=offset, ap=[list(d) for d in dims])


def build_body(nc, tc, x_in, out_fc, w_params, wfcT, gb, sel_in,
               a_act, convout, pooled, h_bin, cc_in, cc_out, st_dram):
    import contextlib
    ctx = contextlib.ExitStack()
    singles = ctx.enter_context(tc.tile_pool(name="singles", bufs=1))
    stats_pool = ctx.enter_context(tc.tile_pool(name="stats", bufs=1))
    small = ctx.enter_context(tc.tile_pool(name="small", bufs=4))

    # ---- load all weights once ----
    w_sb = []
    for li in range(6):
        cin, cout, kh, kw, hin, ps, sg, ch, oh = conv_dims(li)
        tiles = []
        for si, (k0, nkh) in enumerate(kh_splits(cin, kh)):
            k = sg * cin * nkh
            wt = singles.tile([k, kw, sg * cout], BF16, tag=f"w{li}_{si}")
            nc.sync.dma_start(out=wt, in_=w_params[li][si][:, :, :])
            tiles.append(wt)
        w_sb.append(tiles)
    wfc_sb = singles.tile([128, 2, 50], BF16, tag="wfc")
    nc.sync.dma_start(out=wfc_sb, in_=dmaap(wfcT, 0, [[50, 128], [6400, 2], [1, 50]]))
    sel_sb = singles.tile([128, 219], F32, tag="sel")
    nc.sync.dma_start(out=sel_sb, in_=sel_in[:, :])
    gb_sb = [singles.tile([c, 2], F32, tag=f"gb{i}", name=f"gb_sb{i}")
             for i, c in zip(range(7), [3, 8, 16, 32, 32, 64, 64])]
    for i in range(7):
        nc.sync.dma_start(out=gb_sb[i], in_=gb[i][:, :])
    eps_sb = singles.tile([64, 1], F32, tag="eps")
    nc.vector.memset(eps_sb, EPS)

    # HAM ignition: only K=128 matmuls un-throttle the PE clock
    # (1.2 -> 2.4 GHz); once warm, small-K conv matmuls sustain it. Fire a
    # ~5us K=128 burst at the start of each conv phase (the PE re-throttles
    # during the DVE-only pool/stats passes between layers).
    w_warm = singles.tile([128, 256], BF16, tag="w_warm")
    nc.vector.memset(w_warm, 1.0)
    heat_pool = ctx.enter_context(tc.tile_pool(name="heat", bufs=1, space="PSUM"))
    heat_ps = heat_pool.tile([64, 256], F32, tag="heat")

    def ignite(n=24):
        for _ in range(n):
            nc.tensor.matmul(heat_ps, w_warm[:, 0:64], w_warm,
                             start=True, stop=True)

    sel_off = [0, 3, 11, 27, 59, 91, 155]  # column offsets per layer in sel_sb
    sel_w = [3, 8, 16, 32, 32, 64, 64]

    def stats_finish(li, acc, n_tot):
        """acc: [128, 2] sbuf partials -> allreduce -> s,t in st_dram[li].
        li: 0..6 (bn index). Returns nothing; st_dram[li] written."""
        c = sel_w[li]
        with tc.tile_pool(name=f"stp{li}", bufs=1, space="PSUM") as psp:
            ps = psp.tile([c, 2], F32)
            nc.tensor.matmul(ps, sel_sb[:, sel_off[li]:sel_off[li] + c], acc,
                             start=True, stop=True)
            red = small.tile([c, 2], F32, tag="red")
            nc.vector.tensor_copy(red, ps)
        nc.sync.dma_start(out=cc_in[li][:, :], in_=red)
        nc.gpsimd.collective_compute(
            "AllReduce", mybir.AluOpType.add,
            replica_groups=[list(range(N_CORES))],
            ins=[cc_in[li][:, :]], outs=[cc_out[li][:, :]])
        tot = small.tile([c, 2], F32, tag="tot")
        nc.sync.dma_start(out=tot, in_=cc_out[li][:, :])
        # mean/ex2
        mv = small.tile([c, 2], F32, tag="mv")
        nc.scalar.activation(mv, tot, mybir.ActivationFunctionType.Copy,
                             bias=0.0, scale=1.0 / n_tot)
        m2 = small.tile([c, 1], F32, tag="m2")
        nc.scalar.square(m2, mv[:, 0:1])
        var = small.tile([c, 1], F32, tag="var")
        nc.vector.tensor_sub(var, mv[:, 1:2], m2)
        sd = small.tile([c, 1], F32, tag="sd")
        nc.scalar.activation(sd, var, mybir.ActivationFunctionType.Sqrt,
                             bias=eps_sb[0:c, :], scale=1.0)
        rstd = small.tile([c, 1], F32, tag="rstd")
        nc.vector.reciprocal(rstd, sd)
        st = small.tile([c, 2], F32, tag="st")
        nc.vector.tensor_mul(st[:, 0:1], gb_sb[li][:, 0:1], rstd)       # s
        ms = small.tile([c, 1], F32, tag="ms")
        nc.vector.tensor_mul(ms, mv[:, 0:1], st[:, 0:1])
        nc.vector.tensor_sub(st[:, 1:2], gb_sb[li][:, 1:2], ms)         # t
        nc.sync.dma_start(out=st_dram[li][:, :], in_=st)

    # ================= bn1 over x, binarize -> act0 =================
    # partition p = c*32 + n*4 + yc  (c:3, n:8, yc:4 chunks of 56 rows),
    # free chunked in 4 pieces of 14 rows to bound SBUF usage.
    XF = 14 * 224
    acc0 = stats_pool.tile([128, 2], F32, tag="acc0")
    nc.vector.memset(acc0, 0.0)
    with tc.tile_pool(name="bn1", bufs=2) as bn1p:
        for ci in range(4):
            xt = bn1p.tile([96, XF], F32, tag="xt")
            off = ci * XF
            nc.sync.dma_start(out=xt, in_=dmaap(
                x_in, off, [[4 * XF, 96], [1, XF]]))
            r0 = bn1p.tile([96, 1], F32, tag="r0")
            nc.vector.tensor_reduce(out=r0, in_=xt,
                                    axis=mybir.AxisListType.X,
                                    op=mybir.AluOpType.add)
            nc.vector.tensor_add(acc0[0:96, 0:1], acc0[0:96, 0:1], r0)
            sq0 = bn1p.tile([96, XF], BF16, tag="sq0")
            r1 = bn1p.tile([96, 1], F32, tag="r1")
            nc.scalar.activation(sq0, xt, mybir.ActivationFunctionType.Square,
                                 accum_out=r1)
            nc.vector.tensor_add(acc0[0:96, 1:2], acc0[0:96, 1:2], r1)
        stats_finish(0, acc0, 64 * 224 * 224)
        # broadcast s,t -> [96, 2] and sign
        st0 = bn1p.tile([96, 2], F32, tag="st0")
        st0b = st0[:, :]
        for yc in range(4):
            # partitions p = n*12 + c*4 + yc for (n, c); partition pitch = 2
            dst = bass.AP(tensor=st0b.tensor, offset=st0b.offset + yc * st0b.ap[0][0],
                          ap=[[4 * st0b.ap[0][0], 24], [1, 2]])
            nc.sync.dma_start(out=dst,
                              in_=dmaap(st_dram[0], 0, [[0, 8], [2, 3], [1, 2]]))
        for ci in range(4):
            xt = bn1p.tile([96, XF], F32, tag="xt")
            off = ci * XF
            nc.sync.dma_start(out=xt, in_=dmaap(
                x_in, off, [[4 * XF, 96], [1, XF]]))
            bin0 = bn1p.tile([96, XF], BF16, tag="bin0")
            nc.scalar.activation(bin0, xt, mybir.ActivationFunctionType.Sign,
                                 bias=st0[:, 1:2], scale=st0[:, 0:1])
            nc.sync.dma_start(
                out=dmaap(a_act[0], off, [[4 * XF, 96], [1, XF]]),
                in_=bin0)
            if ci == 3:
                nc.gpsimd.tensor_copy(w_warm[0:1, 0:1], bin0[0:1, 0:1])
                ignite()

    # ================= conv layers =================
    for li in range(6):
        cin, cout, kh, kw, hin, ps, sg, ch, oh = conv_dims(li)
        cw = ch
        win = hin
        splits = kh_splits(cin, kh)
        m = sg * cout
        ry = psum_rows(cw)
        n_strips0 = 4 if sg * cout <= 32 else 2
        ry4 = ry * n_strips0
        yb = max(ry4, 16) if ch > 16 else ch
        yb = (yb // ry4) * ry4 if ch > 16 else ch

        with tc.tile_pool(name=f"c{li}R", bufs=3) as rpool, \
             tc.tile_pool(name=f"c{li}P", bufs=6, space="PSUM") as pspool, \
             tc.tile_pool(name=f"c{li}S", bufs=3) as stpool:
            for g in range(NS // sg):
                n0 = g * sg
                y0 = 0
                while y0 < ch:
                    nb_rows = min(yb, ch - y0)
                    # load R tiles (kh-replicated input rows)
                    rts = []
                    for si, (k0, nkh) in enumerate(splits):
                        kdim = sg * cin * nkh
                        rt = rpool.tile([kdim, nb_rows, win], BF16,
                                        tag=f"R{si}")
                        src = dmaap(
                            a_act[li],
                            n0 * cin * win * win + (y0 + k0) * win,
                            [[win * win, sg * cin],
                             [win, nkh], [1, nb_rows * win]])
                        nc.sync.dma_start(out=rt, in_=src)
                        rts.append(rt)

                    n_strips = 4 if m <= 32 else 2
                    s_off = 32 if m <= 32 else 64
                    ry4 = ry * n_strips
                    ry0 = 0
                    while ry0 < nb_rows:
                        strips = []
                        for j in range(n_strips):
                            r0 = ry0 + j * ry
                            nr = min(ry, nb_rows - r0)
                            if nr > 0:
                                strips.append((j, r0, nr))
                        big = pspool.tile([128, ry, cw], F32, tag="ps")
                        nmm = len(splits) * kw
                        imm = 0
                        for si, (k0, nkh) in enumerate(splits):
                            for kx in range(kw):
                                imm += 1
                                for (j, r0, nr) in strips:
                                    nc.tensor.matmul(
                                        big[s_off * j:s_off * j + m, 0:nr, :],
                                        w_sb[li][si][:, kx, :],
                                        rts[si][:, r0:r0 + nr, kx:kx + cw],
                                        start=(imm == 1),
                                        stop=(imm == nmm),
                                        tile_position=(0, s_off * j))
                        stg = stpool.tile([128, ry, cw], F16, tag="stg")
                        if (ry0 // ry4) % 2 == 0:
                            nc.scalar.copy(stg, big)
                        else:
                            nc.vector.tensor_copy(stg, big)
                        rows_here = sum(nr for _, _, nr in strips)
                        if all(nr == ry for _, _, nr in strips):
                            # one DMA: partitions (j-strip, s*cout+co) with holes
                            sb = stg[:, :, :]
                            srcap = bass.AP(
                                tensor=sb.tensor, offset=sb.offset,
                                ap=[[sb.ap[0][0] * s_off, len(strips)],
                                    [sb.ap[0][0], m], [1, ry * cw]])
                            dst = dmaap(
                                convout[li],
                                n0 * cout * ch * cw + (y0 + ry0) * cw,
                                [[ry * cw, len(strips)],
                                 [ch * cw, sg * cout], [1, ry * cw]])
                            nc.sync.dma_start(out=dst, in_=srcap)
                        else:
                            for (j, r0, nr) in strips:
                                dst = dmaap(
                                    convout[li],
                                    n0 * cout * ch * cw + (y0 + r0) * cw,
                                    [[ch * cw, sg * cout], [1, nr * cw]])
                                nc.sync.dma_start(
                                    out=dst, in_=stg[s_off * j:s_off * j + m,
                                                     0:nr, :])
                        ry0 += ry4
                    y0 += nb_rows

        # ---- pool + stats pass ----
        nb = min(NS, 128 // cout)      # samples per pass
        p_used = cout * nb
        ox = oh
        acc = stats_pool.tile([128, 2], F32, tag=f"acc{li + 1}")
        nc.vector.memset(acc, 0.0)
        # chunk pooled rows
        oyc = oh
        while (ps * (oyc - 1) + 3) * cw * 2 > 12000:
            oyc = (oyc + 1) // 2
        max_rows = ps * (oyc - 1) + 3
        with tc.tile_pool(name=f"p{li}", bufs=2) as ppool:
            for pa in range(NS // nb):
                npa = pa * nb
                oy0 = 0
                while oy0 < oh:
                    noy = min(oyc, oh - oy0)
                    rows = ps * (noy - 1) + 3
                    ld = ppool.tile([p_used, max_rows, cw], F16, tag="ld")
                    src = dmaap(
                        convout[li],
                        npa * cout * ch * cw + (oy0 * ps) * cw,
                        [[ch * cw, nb * cout], [1, rows * cw]])
                    nc.sync.dma_start(out=ld[:, 0:rows, :], in_=src)
                    ldb = ld[:, :, :]

                    def ldv(dx):
                        return bass.AP(
                            tensor=ldb.tensor, offset=ldb.offset + dx,
                            ap=[[ldb.ap[0][0], p_used], [cw, rows], [ps, ox]])

                    ph = ppool.tile([p_used, max_rows, ox], F16, tag="ph")
                    nc.vector.tensor_max(ph[:, 0:rows, :], ldv(0), ldv(1))
                    nc.vector.tensor_max(ph[:, 0:rows, :], ph[:, 0:rows, :], ldv(2))
                    phb = ph[:, :, :]

                    def phv(dy):
                        return bass.AP(
                            tensor=phb.tensor, offset=phb.offset + dy * ox,
                            ap=[[phb.ap[0][0], p_used], [ps * ox, noy], [1, ox]])

                    pv = ppool.tile([p_used, oyc, ox], F16, tag="pv")
                    nc.vector.tensor_max(pv[:, 0:noy, :], phv(0), phv(1))
                    nc.vector.tensor_max(pv[:, 0:noy, :], pv[:, 0:noy, :], phv(2))
                    # stats
                    red = ppool.tile([p_used, 1], F32, tag="redp")
                    nc.vector.tensor_reduce(out=red, in_=pv[:, 0:noy, :],
                                            axis=mybir.AxisListType.XY,
                                            op=mybir.AluOpType.add)
                    nc.vector.tensor_add(acc[0:p_used, 0:1], acc[0:p_used, 0:1], red)
                    sqs = ppool.tile([p_used, oyc, ox], BF16, tag="sqs")
                    red2 = ppool.tile([p_used, 1], F32, tag="redq")
                    nc.scalar.activation(sqs[:, 0:noy, :], pv[:, 0:noy, :],
                                         mybir.ActivationFunctionType.Square,
                                         accum_out=red2)
                    nc.vector.tensor_add(acc[0:p_used, 1:2], acc[0:p_used, 1:2], red2)
                    # store pooled
                    dst = dmaap(
                        pooled[li],
                        npa * cout * oh * ox + oy0 * ox,
                        [[oh * ox, nb * cout], [1, noy * ox]])
                    nc.sync.dma_start(out=dst, in_=pv[:, 0:noy, :])
                    oy0 += noy

        stats_finish(li + 1, acc, 64 * oh * oh)

        # ---- binarize pass: pooled -> a_act[li+1] (or h_bin for last) ----
        dst_t = a_act[li + 1] if li < 5 else h_bin
        stb = stats_pool.tile([128, 2], F32, tag=f"stb{li + 1}")
        nc.sync.dma_start(out=stb[0:p_used, :],
                          in_=dmaap(st_dram[li + 1], 0, [[0, nb], [2, cout], [1, 2]]))
        f_all = oh * oh
        with tc.tile_pool(name=f"b{li}", bufs=3) as bpool:
            for pa in range(NS // nb):
                npa = pa * nb
                f0 = 0
                fchunk = min(f_all, 4096)
                while f0 < f_all:
                    nf = min(fchunk, f_all - f0)
                    ld = bpool.tile([p_used, fchunk], F16, tag="bl")
                    nc.sync.dma_start(
                        out=ld[:, 0:nf],
                        in_=dmaap(pooled[li], npa * cout * f_all + f0,
                                  [[f_all, nb * cout], [1, nf]]))
                    bo = bpool.tile([p_used, fchunk], BF16, tag="bo")
                    nc.scalar.activation(bo[:, 0:nf], ld[:, 0:nf],
                                         mybir.ActivationFunctionType.Sign,
                                         bias=stb[0:p_used, 1:2],
                                         scale=stb[0:p_used, 0:1])
                    nc.sync.dma_start(
                        out=dmaap(dst_t, npa * cout * f_all + f0,
                                  [[f_all, nb * cout], [1, nf]]),
                        in_=bo[:, 0:nf])
                    f0 += nf
                    if pa == NS // nb - 1 and f0 >= f_all and li < 5:
                        nc.gpsimd.tensor_copy(w_warm[0:1, 0:1], bo[0:1, 0:1])
                        ignite()

    # ================= FC =================
    with tc.tile_pool(name="fc", bufs=1) as fcp, \
         tc.tile_pool(name="fcps", bufs=1, space="PSUM") as fcps:
        ht = [fcp.tile([128, NS], BF16, tag=f"ht{i}", name=f"ht{i}") for i in range(2)]
        for i in range(2):
            nc.sync.dma_start(out=ht[i], in_=dmaap(
                h_bin, i * 128, [[1, 128], [256, NS]]))
        ps = fcps.tile([NS, 50], F32)
        nc.tensor.matmul(ps, ht[0], wfc_sb[:, 0, :], start=True, stop=False)
        nc.tensor.matmul(ps, ht[1], wfc_sb[:, 1, :], start=False, stop=True)
        fo = fcp.tile([NS, 50], F32, tag="fo")
        nc.vector.tensor_copy(fo, ps)
        nc.sync.dma_start(out=out_fc[:, :], in_=fo)

    ctx.close()


# ---------------------------------------------------------------------------
# entry point
# ---------------------------------------------------------------------------

_cache = {}


def _get_nc():
    if "nc" not in _cache:
        nc = bacc.Bacc("TRN2", target_bir_lowering=False, num_devices=N_CORES)
        _cache["nc"] = build(nc)
    return _cache["nc"]


def _shared_maps(inputs):
    shared = {}
    for li in range(6):
        w = np.asarray(inputs[f"w{li + 1}"], np.float32)
        cin, cout, kh, kw, hin, ps, sg, ch, oh = conv_dims(li)
        for si, arr in enumerate(pack_conv_weights(w, sg)):
            shared[f"w{li + 1}_{si}"] = arr
    shared["wfcT"] = ml_bf16(np.sign(np.asarray(inputs["wfc"], np.float32)).T.copy())
    for i, c in zip(range(1, 8), [3, 8, 16, 32, 32, 64, 64]):
        shared[f"gb{i}"] = np.stack([np.asarray(inputs[f"g{i}"], np.float32),
                                     np.asarray(inputs[f"b{i}"], np.float32)],
                                    axis=1).copy()
    sels = []
    for c in [3, 8, 16, 32, 32, 64, 64]:
        if c == 3:
            sel = np.zeros((128, 3), np.float32)
            for ch_ in range(3):
                for n in range(NS):
                    for yc in range(4):
                        sel[n * 12 + ch_ * 4 + yc, ch_] = 1.0
        else:
            sel = make_selector(c, min(NS, 128 // c))
        sels.append(sel)
    shared["sels"] = np.concatenate(sels, axis=1).copy()
    return shared


def _in_maps(inputs):
    x = np.asarray(inputs["x"], np.float32)
    if "shared" not in _cache:
        _cache["shared"] = _shared_maps(inputs)
    shared = _cache["shared"]
    in_maps = []
    for c in range(N_CORES):
        m = dict(shared)
        m["x"] = np.ascontiguousarray(x[c * NS:(c + 1) * NS])
        in_maps.append(m)
    return in_maps


def _make_runner(nc):
    """Cached jitted SPMD executor (run_bass_via_pjrt retraces per call)."""
    import jax
    from jax.experimental.shard_map import shard_map
    from jax.sharding import Mesh, PartitionSpec
    from concourse import bass2jax, mybir as _mb

    bass2jax.install_neuronx_cc_hook()
    partition_name = (nc.partition_id_tensor.name
                      if nc.partition_id_tensor else None)
    in_names, out_names, out_avals, zero_outs = [], [], [], []
    for alloc in nc.m.functions[0].allocations:
        if not isinstance(alloc, _mb.MemoryLocationSet):
            continue
        name = alloc.memorylocations[0].name
        if alloc.kind == "ExternalInput":
            if name != partition_name:
                in_names.append(name)
        elif alloc.kind == "ExternalOutput":
            out_names.append(name)
            shape = tuple(alloc.tensor_shape)
            dtype = _mb.dt.np(alloc.dtype)
            out_avals.append(jax.core.ShapedArray(shape, dtype))
            zero_outs.append(np.zeros((N_CORES * shape[0],) + shape[1:], dtype))
    n_params = len(in_names)
    all_names = in_names + out_names
    if partition_name is not None:
        all_names = all_names + [partition_name]
    donate = tuple(range(n_params, n_params + len(out_names)))

    def _body(*args):
        operands = list(args)
        if partition_name is not None:
            operands.append(bass2jax.partition_id_tensor())
        outs = bass2jax._bass_exec_p.bind(
            *operands, out_avals=tuple(out_avals), in_names=tuple(all_names),
            out_names=tuple(out_names), lowering_input_output_aliases=(),
            sim_require_finite=True, sim_require_nnan=True, nc=nc)
        return tuple(outs)

    devices = jax.devices()[:N_CORES]
    mesh = Mesh(np.asarray(devices), ("core",))
    nin = n_params + len(out_names)
    sharded = jax.jit(
        shard_map(_body, mesh=mesh, in_specs=(PartitionSpec("core"),) * nin,
                  out_specs=(PartitionSpec("core"),) * len(out_names),
                  check_rep=False),
        donate_argnums=donate, keep_unused=True)

    def run(in_maps):
        concat_in = [np.concatenate([np.asarray(in_maps[c][n])
                                     for c in range(N_CORES)], axis=0)
                     for n in in_names]
        out_arrs = sharded(*concat_in, *zero_outs)
        return np.asarray(out_arrs[out_names.index("out")])

    return run


def kernel(**inputs):
    nc = _get_nc()
    if "runner" not in _cache:
        _cache["runner"] = _make_runner(nc)
    return _cache["runner"](_in_maps(inputs))


def profile(**inputs):
    """Run with NTFF tracing; returns (exec_time_ns, results) or None."""
    import types
    try:
        from antenv import axon_hooks
    except ImportError:
        import antenv
        axon_hooks = types.ModuleType("antenv.axon_hooks")
        axon_hooks._hook = None
        def _set(h):
            axon_hooks._hook = h
        def _get():
            return axon_hooks._hook
        axon_hooks.set_axon_ntff_profile_hook = _set
        axon_hooks.get_axon_ntff_profile_hook = _get
        sys.modules["antenv.axon_hooks"] = axon_hooks
        antenv.axon_hooks = axon_hooks
    if axon_hooks.get_axon_ntff_profile_hook() is None:
        sys.path.insert(0, "/root/.axon_site")
        from trn_agent_boot.trn_boot import _ntff_profile_via_ctypes
        hook = _ntff_profile_via_ctypes("/opt/axon/libaxon_pjrt.so")
        if hook is not None:
            axon_hooks.set_axon_ntff_profile_hook(hook)
    import concourse.bass_utils as bu
    bu.upload_artifacts = lambda tmpdir: "(skipped)"
    nc = _get_nc()
    res = run_bass_kernel_spmd(nc, _in_maps(inputs), list(range(N_CORES)),
                               trace=True)
    return res.exec_time_ns


if __name__ == "__main__":
    rng = np.random.default_rng(0)
    fake = {"x": rng.standard_normal((64, 3, 224, 224), dtype=np.float32)}
    for i, (ci, co, kh, kw) in zip(range(1, 7),
                                   [(3, 8, 11, 11), (8, 16, 7, 7), (16, 32, 5, 5),
                                    (32, 32, 5, 5), (32, 64, 5, 5), (64, 64, 3, 3)]):
        fake[f"w{i}"] = rng.standard_normal((co, ci, kh, kw), dtype=np.float32) * 0.1
    for i, c in zip(range(1, 8), [3, 8, 16, 32, 32, 64, 64]):
        fake[f"g{i}"] = np.ones(c, np.float32)
        fake[f"b{i}"] = np.zeros(c, np.float32)
    fake["wfc"] = rng.standard_normal((50, 256), dtype=np.float32) * 0.1
    out = kernel(**fake)
    print("out", out.shape, out[0, :5])


# revision 23
# speedup vs baseline: 1.9276x; 1.0318x over previous
"""Trainium2 Bass kernel for BinSpecCNN: 7x (BN -> sign) + 6 binary convs
with maxpool + final FC, data-parallel over 8 NeuronCores (8 samples each).

Strategy per core:
  - bn_i + sign fused as ACT Sign(scale*x+bias) passes; batch statistics
    (per-channel sum/sumsq) all-reduced across cores per layer.
  - convs: bf16 matmuls (inputs/weights are exactly +-1 in bf16), contraction
    dim = (sample-blockdiag x Cin x KH) on partitions, KW looped via
    free-dim-shifted rhs with PSUM accumulation. Input rows are KH-replicated
    into SBUF by DMA.
  - maxpool: two InstPool stages (horizontal window then vertical) on f16.
  - conv outputs (integer-valued, |v| <= 800) stored in f16 (exact).
"""

import sys

sys.path.insert(0, "/opt/trn_rl_repo")

import numpy as np

import concourse.bass as bass
import concourse.mybir as mybir
import concourse.tile as tile
from concourse import bacc
from concourse.bass_utils import run_bass_kernel_spmd

F32 = mybir.dt.float32
F16 = mybir.dt.float16
BF16 = mybir.dt.bfloat16

N_CORES = 8
NS = 8  # samples per core
EPS = 1e-5

# (Cin, Cout, KH, KW, Hin, pool_stride, sample_group)
CONVS = [
    (3, 8, 11, 11, 224, 1, 2),
    (8, 16, 7, 7, 212, 2, 2),
    (16, 32, 5, 5, 102, 2, 1),
    (32, 32, 5, 5, 48, 2, 1),
    (32, 64, 5, 5, 21, 2, 1),
    (64, 64, 3, 3, 8, 2, 1),
]


def conv_dims(li):
    cin, cout, kh, kw, hin, ps, sg = CONVS[li]
    ch = hin - kh + 1            # conv output height/width
    oh = (ch - 3) // ps + 1      # pooled output height/width
    return cin, cout, kh, kw, hin, ps, sg, ch, oh


def kh_splits(cin, kh):
    """Split KH taps so each split's (cin*nkh) <= 128 partitions."""
    max_kh = 128 // cin
    splits = []
    k0 = 0
    while k0 < kh:
        n = min(max_kh, kh - k0)
        splits.append((k0, n))
        k0 += n
    return splits


# rows-of-conv-output per psum tile (free dim = ry*cw <= 512)
def psum_rows(cw):
    return max(1, min(512 // cw, 8))


# ---------------------------------------------------------------------------
# host-side weight packing
# ---------------------------------------------------------------------------

def pack_conv_weights(w, sg):
    """w: [Cout, Cin, KH, KW] float32 -> per-khsplit lhsT arrays
    [K, KW, M] bf16 with K=(s, c, kh), M=(s, co) block-diagonal over s."""
    wb = np.sign(w).astype(np.float32)
    cout, cin, kh, kw = w.shape
    out = []
    for (k0, nkh) in kh_splits(cin, kh):
        m = sg * cout
        k = sg * cin * nkh
        arr = np.zeros((k, kw, m), np.float32)
        for s in range(sg):
            for c in range(cin):
                for kk in range(nkh):
                    p = (s * cin + c) * nkh + kk
                    # lhsT[p, kw, s*cout + co] = wb[co, c, k0+kk, kw]
                    arr[p, :, s * cout:(s + 1) * cout] = wb[:, c, k0 + kk, :].T
        out.append(ml_bf16(arr))
    return out


def ml_bf16(a):
    import ml_dtypes
    return np.asarray(a, dtype=np.float32).astype(ml_dtypes.bfloat16)


def ml_f16(a):
    return np.asarray(a, dtype=np.float16)


def make_selector(c, nb):
    """[128, c] selector: partition p=(g*c+ch) -> column ch."""
    sel = np.zeros((128, c), np.float32)
    for ch in range(c):
        for g in range(nb):
            sel[g * c + ch, ch] = 1.0
    return sel


# ---------------------------------------------------------------------------
# device kernel builder
# ---------------------------------------------------------------------------

def build(nc):
    x_in = nc.declare_dram_parameter("x", [NS, 3, 224, 224], F32, isOutput=False)
    out_fc = nc.declare_dram_parameter("out", [NS, 50], F32, isOutput=True)

    # weights / constants (identical on all cores)
    w_params = []   # per layer: list of lhsT dram handles
    for li in range(6):
        cin, cout, kh, kw, hin, ps, sg, ch, oh = conv_dims(li)
        splits = kh_splits(cin, kh)
        handles = []
        for si, (k0, nkh) in enumerate(splits):
            k = sg * cin * nkh
            handles.append(nc.declare_dram_parameter(
                f"w{li + 1}_{si}", [k, kw, sg * cout], BF16, isOutput=False))
        w_params.append(handles)
    wfcT = nc.declare_dram_parameter("wfcT", [256, 50], BF16, isOutput=False)

    gb = [nc.declare_dram_parameter(f"gb{i}", [c, 2], F32, isOutput=False)
          for i, c in zip(range(1, 8), [3, 8, 16, 32, 32, 64, 64])]
    sel_in = nc.declare_dram_parameter("sels", [128, 3 + 8 + 16 + 32 + 32 + 64 + 64],
                                       F32, isOutput=False)

    # internal DRAM
    a_act = []   # binarized activations bf16, input of conv li
    for li in range(6):
        cin, cout, kh, kw, hin, ps, sg, ch, oh = conv_dims(li)
        a_act.append(nc.dram_tensor(f"act{li}", [NS, cin, hin, hin], BF16))
    convout = []
    pooled = []
    for li in range(6):
        cin, cout, kh, kw, hin, ps, sg, ch, oh = conv_dims(li)
        convout.append(nc.dram_tensor(f"conv{li}", [NS, cout, ch, ch], F16))
        pooled.append(nc.dram_tensor(f"pool{li}", [NS, cout, oh, oh], F16))
    h_bin = nc.dram_tensor("h_bin", [NS, 64, 2, 2], BF16)  # final binarized

    cc_in = [nc.dram_tensor(f"cc_in{i}", [c, 2], F32)
             for i, c in zip(range(7), [3, 8, 16, 32, 32, 64, 64])]
    cc_out = [nc.dram_tensor(f"cc_out{i}", [c, 2], F32, addr_space="Shared")
              for i, c in zip(range(7), [3, 8, 16, 32, 32, 64, 64])]
    st_dram = [nc.dram_tensor(f"st{i}", [c, 2], F32)
               for i, c in zip(range(7), [3, 8, 16, 32, 32, 64, 64])]

    with tile.TileContext(nc) as tc:
        build_body(nc, tc, x_in, out_fc, w_params, wfcT, gb, sel_in,
                   a_act, convout, pooled, h_bin, cc_in, cc_out, st_dram)
    nc.finalize()
    return nc


def dmaap(t, offset, dims):
    """AP over dram tensor handle t with element offset and [step,count] dims."""
    base = t[tuple(slice(None) for _ in t.shape)]
    return bass.AP(tensor=base.tensor, offset=offset, ap=[list(d) for d in dims])


def build_body(nc, tc, x_in, out_fc, w_params, wfcT, gb, sel_in,
               a_act, convout, pooled, h_bin, cc_in, cc_out, st_dram):
    import contextlib
    ctx = contextlib.ExitStack()
    singles = ctx.enter_context(tc.tile_pool(name="singles", bufs=1))
    stats_pool = ctx.enter_context(tc.tile_pool(name="stats", bufs=1))
    small = ctx.enter_context(tc.tile_pool(name="small", bufs=4))

    # ---- load all weights once ----
    w_sb = []
    for li in range(6):
        cin, cout, kh, kw, hin, ps, sg, ch, oh = conv_dims(li)
        tiles = []
        for si, (k0, nkh) in enumerate(kh_splits(cin, kh)):
            k = sg * cin * nkh
            wt = singles.tile([k, kw, sg * cout], BF16, tag=f"w{li}_{si}")
            nc.sync.dma_start(out=wt, in_=w_params[li][si][:, :, :])
            tiles.append(wt)
        w_sb.append(tiles)
    wfc_sb = singles.tile([128, 2, 50], BF16, tag="wfc")
    nc.sync.dma_start(out=wfc_sb, in_=dmaap(wfcT, 0, [[50, 128], [6400, 2], [1, 50]]))
    sel_sb = singles.tile([128, 219], F32, tag="sel")
    nc.sync.dma_start(out=sel_sb, in_=sel_in[:, :])
    gb_sb = [singles.tile([c, 2], F32, tag=f"gb{i}", name=f"gb_sb{i}")
             for i, c in zip(range(7), [3, 8, 16, 32, 32, 64, 64])]
    for i in range(7):
        nc.sync.dma_start(out=gb_sb[i], in_=gb[i][:, :])
    eps_sb = singles.tile([64, 1], F32, tag="eps")
    nc.vector.memset(eps_sb, EPS)

    # HAM ignition: only K=128 matmuls un-throttle the PE clock
    # (1.2 -> 2.4 GHz); once warm, small-K conv matmuls sustain it. Fire a
    # ~5us K=128 burst at the start of each conv phase (the PE re-throttles
    # during the DVE-only pool/stats passes between layers).
    w_warm = singles.tile([128, 256], BF16, tag="w_warm")
    nc.vector.memset(w_warm, 1.0)
    heat_pool = ctx.enter_context(tc.tile_pool(name="heat", bufs=1, space="PSUM"))
    heat_ps = heat_pool.tile([64, 256], F32, tag="heat")

    def ignite(n=24):
        for _ in range(n):
            nc.tensor.matmul(heat_ps, w_warm[:, 0:64], w_warm,
                             start=True, stop=True)

    sel_off = [0, 3, 11, 27, 59, 91, 155]  # column offsets per layer in sel_sb
    sel_w = [3, 8, 16, 32, 32, 64, 64]

    def stats_finish(li, acc, n_tot):
        """acc: [128, 2] sbuf partials -> allreduce -> s,t in st_dram[li].
        li: 0..6 (bn index). Returns nothing; st_dram[li] written."""
        c = sel_w[li]
        with tc.tile_pool(name=f"stp{li}", bufs=1, space="PSUM") as psp:
            ps = psp.tile([c, 2], F32)
            nc.tensor.matmul(ps, sel_sb[:, sel_off[li]:sel_off[li] + c], acc,
                             start=True, stop=True)
            red = small.tile([c, 2], F32, tag="red")
            nc.vector.tensor_copy(red, ps)
        nc.sync.dma_start(out=cc_in[li][:, :], in_=red)
        nc.gpsimd.collective_compute(
            "AllReduce", mybir.AluOpType.add,
            replica_groups=[list(range(N_CORES))],
            ins=[cc_in[li][:, :]], outs=[cc_out[li][:, :]])
        tot = small.tile([c, 2], F32, tag="tot")
        nc.sync.dma_start(out=tot, in_=cc_out[li][:, :])
        # mean/ex2
        mv = small.tile([c, 2], F32, tag="mv")
        nc.scalar.activation(mv, tot, mybir.ActivationFunctionType.Copy,
                             bias=0.0, scale=1.0 / n_tot)
        m2 = small.tile([c, 1], F32, tag="m2")
        nc.scalar.square(m2, mv[:, 0:1])
        var = small.tile([c, 1], F32, tag="var")
        nc.vector.tensor_sub(var, mv[:, 1:2], m2)
        sd = small.tile([c, 1], F32, tag="sd")
        nc.scalar.activation(sd, var, mybir.ActivationFunctionType.Sqrt,
                             bias=eps_sb[0:c, :], scale=1.0)
        rstd = small.tile([c, 1], F32, tag="rstd")
        nc.vector.reciprocal(rstd, sd)
        st = small.tile([c, 2], F32, tag="st")
        nc.vector.tensor_mul(st[:, 0:1], gb_sb[li][:, 0:1], rstd)       # s
        ms = small.tile([c, 1], F32, tag="ms")
        nc.vector.tensor_mul(ms, mv[:, 0:1], st[:, 0:1])
        nc.vector.tensor_sub(st[:, 1:2], gb_sb[li][:, 1:2], ms)         # t
        nc.sync.dma_start(out=st_dram[li][:, :], in_=st)

    # ================= bn1 over x, binarize -> act0 =================
    # partition p = c*32 + n*4 + yc  (c:3, n:8, yc:4 chunks of 56 rows),
    # free chunked in 4 pieces of 14 rows to bound SBUF usage.
    XF = 14 * 224
    acc0 = stats_pool.tile([128, 2], F32, tag="acc0")
    nc.vector.memset(acc0, 0.0)
    with tc.tile_pool(name="bn1", bufs=2) as bn1p:
        for ci in range(4):
            xt = bn1p.tile([96, XF], F32, tag="xt")
            off = ci * XF
            nc.sync.dma_start(out=xt, in_=dmaap(
                x_in, off, [[4 * XF, 96], [1, XF]]))
            r0 = bn1p.tile([96, 1], F32, tag="r0")
            nc.vector.tensor_reduce(out=r0, in_=xt,
                                    axis=mybir.AxisListType.X,
                                    op=mybir.AluOpType.add)
            nc.vector.tensor_add(acc0[0:96, 0:1], acc0[0:96, 0:1], r0)
            sq0 = bn1p.tile([96, XF], BF16, tag="sq0")
            r1 = bn1p.tile([96, 1], F32, tag="r1")
            nc.scalar.activation(sq0, xt, mybir.ActivationFunctionType.Square,
                                 accum_out=r1)
            nc.vector.tensor_add(acc0[0:96, 1:2], acc0[0:96, 1:2], r1)
        stats_finish(0, acc0, 64 * 224 * 224)
        # broadcast s,t -> [96, 2] and sign
        st0 = bn1p.tile([96, 2], F32, tag="st0")
        st0b = st0[:, :]
        for yc in range(4):
            # partitions p = n*12 + c*4 + yc for (n, c); partition pitch = 2
            dst = bass.AP(tensor=st0b.tensor, offset=st0b.offset + yc * st0b.ap[0][0],
                          ap=[[4 * st0b.ap[0][0], 24], [1, 2]])
            nc.sync.dma_start(out=dst,
                              in_=dmaap(st_dram[0], 0, [[0, 8], [2, 3], [1, 2]]))
        for ci in range(4):
            xt = bn1p.tile([96, XF], F32, tag="xt")
            off = ci * XF
            nc.sync.dma_start(out=xt, in_=dmaap(
                x_in, off, [[4 * XF, 96], [1, XF]]))
            bin0 = bn1p.tile([96, XF], BF16, tag="bin0")
            nc.scalar.activation(bin0, xt, mybir.ActivationFunctionType.Sign,
                                 bias=st0[:, 1:2], scale=st0[:, 0:1])
            nc.sync.dma_start(
                out=dmaap(a_act[0], off, [[4 * XF, 96], [1, XF]]),
                in_=bin0)
            if ci == 3:
                nc.gpsimd.tensor_copy(w_warm[0:1, 0:1], bin0[0:1, 0:1])
                ignite()

    # ================= conv layers =================
    for li in range(6):
        cin, cout, kh, kw, hin, ps, sg, ch, oh = conv_dims(li)
        cw = ch
        win = hin
        splits = kh_splits(cin, kh)
        m = sg * cout
        ry = psum_rows(cw)
        n_strips0 = 4 if sg * cout <= 32 else 2
        ry4 = ry * n_strips0
        yb = max(ry4, 16) if ch > 16 else ch
        yb = (yb // ry4) * ry4 if ch > 16 else ch

        with tc.tile_pool(name=f"c{li}R", bufs=3) as rpool, \
             tc.tile_pool(name=f"c{li}P", bufs=6, space="PSUM") as pspool, \
             tc.tile_pool(name=f"c{li}S", bufs=3) as stpool:
            for g in range(NS // sg):
                n0 = g * sg
                y0 = 0
                while y0 < ch:
                    nb_rows = min(yb, ch - y0)
                    # load R tiles (kh-replicated input rows)
                    rts = []
                    for si, (k0, nkh) in enumerate(splits):
                        kdim = sg * cin * nkh
                        rt = rpool.tile([kdim, nb_rows, win], BF16,
                                        tag=f"R{si}")
                        src = dmaap(
                            a_act[li],
                            n0 * cin * win * win + (y0 + k0) * win,
                            [[win * win, sg * cin],
                             [win, nkh], [1, nb_rows * win]])
                        nc.sync.dma_start(out=rt, in_=src)
                        rts.append(rt)

                    n_strips = 4 if m <= 32 else 2
                    s_off = 32 if m <= 32 else 64
                    ry4 = ry * n_strips
                    ry0 = 0
                    while ry0 < nb_rows:
                        strips = []
                        for j in range(n_strips):
                            r0 = ry0 + j * ry
                            nr = min(ry, nb_rows - r0)
                            if nr > 0:
                                strips.append((j, r0, nr))
                        big = pspool.tile([128, ry, cw], F32, tag="ps")
                        nmm = len(splits) * kw
                        imm = 0
                        for si, (k0, nkh) in enumerate(splits):
                            for kx in range(kw):
                                imm += 1
                                for (j, r0, nr) in strips:
                                    nc.tensor.matmul(
                                        big[s_off * j:s_off * j + m, 0:nr, :],
                                        w_sb[li][si][:, kx, :],
                                        rts[si][:, r0:r0 + nr, kx:kx + cw],
                                        start=(imm == 1),
                                        stop=(imm == nmm),
                                        tile_position=(0, s_off * j))
                        stg = stpool.tile([128, ry, cw], F16, tag="stg")
                        if (ry0 // ry4) % 2 == 0:
                            nc.scalar.copy(stg, big)
                        else:
                            nc.vector.tensor_copy(stg, big)
                        rows_here = sum(nr for _, _, nr in strips)
                        if False and all(nr == ry for _, _, nr in strips):
                            # one DMA: partitions (j-strip, s*cout+co) with holes
                            sb = stg[:, :, :]
                            srcap = bass.AP(
                                tensor=sb.tensor, offset=sb.offset,
                                ap=[[sb.ap[0][0] * s_off, len(strips)],
                                    [sb.ap[0][0], m], [1, ry * cw]])
                            dst = dmaap(
                                convout[li],
                                n0 * cout * ch * cw + (y0 + ry0) * cw,
                                [[ry * cw, len(strips)],
                                 [ch * cw, sg * cout], [1, ry * cw]])
                            nc.sync.dma_start(out=dst, in_=srcap)
                        else:
                            for (j, r0, nr) in strips:
                                dst = dmaap(
                                    convout[li],
                                    n0 * cout * ch * cw + (y0 + r0) * cw,
                                    [[ch * cw, sg * cout], [1, nr * cw]])
                                nc.sync.dma_start(
                                    out=dst, in_=stg[s_off * j:s_off * j + m,
                                                     0:nr, :])
                        ry0 += ry4
                    y0 += nb_rows

        # ---- pool + stats pass ----
        nb = min(NS, 128 // cout)      # samples per pass
        p_used = cout * nb
        ox = oh
        acc = stats_pool.tile([128, 2], F32, tag=f"acc{li + 1}")
        nc.vector.memset(acc, 0.0)
        # chunk pooled rows
        oyc = oh
        while (ps * (oyc - 1) + 3) * cw * 2 > 12000:
            oyc = (oyc + 1) // 2
        max_rows = ps * (oyc - 1) + 3
        with tc.tile_pool(name=f"p{li}", bufs=2) as ppool:
            for pa in range(NS // nb):
                npa = pa * nb
                oy0 = 0
                while oy0 < oh:
                    noy = min(oyc, oh - oy0)
                    rows = ps * (noy - 1) + 3
                    ld = ppool.tile([p_used, max_rows, cw], F16, tag="ld")
                    src = dmaap(
                        convout[li],
                        npa * cout * ch * cw + (oy0 * ps) * cw,
                        [[ch * cw, nb * cout], [1, rows * cw]])
                    nc.sync.dma_start(out=ld[:, 0:rows, :], in_=src)
                    ldb = ld[:, :, :]

                    def ldv(dx):
                        return bass.AP(
                            tensor=ldb.tensor, offset=ldb.offset + dx,
                            ap=[[ldb.ap[0][0], p_used], [cw, rows], [ps, ox]])

                    ph = ppool.tile([p_used, max_rows, ox], F16, tag="ph")
                    nc.vector.tensor_max(ph[:, 0:rows, :], ldv(0), ldv(1))
                    nc.vector.tensor_max(ph[:, 0:rows, :], ph[:, 0:rows, :], ldv(2))
                    phb = ph[:, :, :]

                    def phv(dy):
                        return bass.AP(
                            tensor=phb.tensor, offset=phb.offset + dy * ox,
                            ap=[[phb.ap[0][0], p_used], [ps * ox, noy], [1, ox]])

                    pv = ppool.tile([p_used, oyc, ox], F16, tag="pv")
                    nc.vector.tensor_max(pv[:, 0:noy, :], phv(0), phv(1))
                    nc.vector.tensor_max(pv[:, 0:noy, :], pv[:, 0:noy, :], phv(2))
                    # stats
                    red = ppool.tile([p_used, 1], F32, tag="redp")
                    nc.vector.tensor_reduce(out=red, in_=pv[:, 0:noy, :],
                                            axis=mybir.AxisListType.XY,
                                            op=mybir.AluOpType.add)
                    nc.vector.tensor_add(acc[0:p_used, 0:1], acc[0:p_used, 0:1], red)
                    sqs = ppool.tile([p_used, oyc, ox], BF16, tag="sqs")
                    red2 = ppool.tile([p_used, 1], F32, tag="redq")
                    nc.scalar.activation(sqs[:, 0:noy, :], pv[:, 0:noy, :],
                                         mybir.ActivationFunctionType.Square,
                                         accum_out=red2)
                    nc.vector.tensor_add(acc[0:p_used, 1:2], acc[0:p_used, 1:2], red2)
                    # store pooled
                    dst = dmaap(
                        pooled[li],
                        npa * cout * oh * ox + oy0 * ox,
                        [[oh * ox, nb * cout], [1, noy * ox]])
                    nc.sync.dma_start(out=dst, in_=pv[:, 0:noy, :])
                    oy0 += noy

        stats_finish(li + 1, acc, 64 * oh * oh)

        # ---- binarize pass: pooled -> a_act[li+1] (or h_bin for last) ----
        dst_t = a_act[li + 1] if li < 5 else h_bin
        stb = stats_pool.tile([128, 2], F32, tag=f"stb{li + 1}")
        nc.sync.dma_start(out=stb[0:p_used, :],
                          in_=dmaap(st_dram[li + 1], 0, [[0, nb], [2, cout], [1, 2]]))
        f_all = oh * oh
        with tc.tile_pool(name=f"b{li}", bufs=3) as bpool:
            for pa in range(NS // nb):
                npa = pa * nb
                f0 = 0
                fchunk = min(f_all, 4096)
                while f0 < f_all:
                    nf = min(fchunk, f_all - f0)
                    ld = bpool.tile([p_used, fchunk], F16, tag="bl")
                    nc.sync.dma_start(
                        out=ld[:, 0:nf],
                        in_=dmaap(pooled[li], npa * cout * f_all + f0,
                                  [[f_all, nb * cout], [1, nf]]))
                    bo = bpool.tile([p_used, fchunk], BF16, tag="bo")
                    nc.scalar.activation(bo[:, 0:nf], ld[:, 0:nf],
                                         mybir.ActivationFunctionType.Sign,
                                         bias=stb[0:p_used, 1:2],
                                         scale=stb[0:p_used, 0:1])
                    nc.sync.dma_start(
                        out=dmaap(dst_t, npa * cout * f_all + f0,
                                  [[f_all, nb * cout], [1, nf]]),
                        in_=bo[:, 0:nf])
                    f0 += nf
                    if pa == NS // nb - 1 and f0 >= f_all and li < 5:
                        nc.gpsimd.tensor_copy(w_warm[0:1, 0:1], bo[0:1, 0:1])
                        ignite()

    # ================= FC =================
    with tc.tile_pool(name="fc", bufs=1) as fcp, \
         tc.tile_pool(name="fcps", bufs=1, space="PSUM") as fcps:
        ht = [fcp.tile([128, NS], BF16, tag=f"ht{i}", name=f"ht{i}") for i in range(2)]
        for i in range(2):
            nc.sync.dma_start(out=ht[i], in_=dmaap(
                h_bin, i * 128, [[1, 128], [256, NS]]))
        ps = fcps.tile([NS, 50], F32)
        nc.tensor.matmul(ps, ht[0], wfc_sb[:, 0, :], start=True, stop=False)
        nc.tensor.matmul(ps, ht[1], wfc_sb[:, 1, :], start=False, stop=True)
        fo = fcp.tile([NS, 50], F32, tag="fo")
        nc.vector.tensor_copy(fo, ps)
        nc.sync.dma_start(out=out_fc[:, :], in_=fo)

    ctx.close()


# ---------------------------------------------------------------------------
# entry point
# ---------------------------------------------------------------------------

_cache = {}


def _get_nc():
    if "nc" not in _cache:
        nc = bacc.Bacc("TRN2", target_bir_lowering=False, num_devices=N_CORES)
        _cache["nc"] = build(nc)
    return _cache["nc"]


def _shared_maps(inputs):
    shared = {}
    for li in range(6):
        w = np.asarray(inputs[f"w{li + 1}"], np.float32)
        cin, cout, kh, kw, hin, ps, sg, ch, oh = conv_dims(li)
        for si, arr in enumerate(pack_conv_weights(w, sg)):
            shared[f"w{li + 1}_{si}"] = arr
    shared["wfcT"] = ml_bf16(np.sign(np.asarray(inputs["wfc"], np.float32)).T.copy())
    for i, c in zip(range(1, 8), [3, 8, 16, 32, 32, 64, 64]):
        shared[f"gb{i}"] = np.stack([np.asarray(inputs[f"g{i}"], np.float32),
                                     np.asarray(inputs[f"b{i}"], np.float32)],
                                    axis=1).copy()
    sels = []
    for c in [3, 8, 16, 32, 32, 64, 64]:
        if c == 3:
            sel = np.zeros((128, 3), np.float32)
            for ch_ in range(3):
                for n in range(NS):
                    for yc in range(4):
                        sel[n * 12 + ch_ * 4 + yc, ch_] = 1.0
        else:
            sel = make_selector(c, min(NS, 128 // c))
        sels.append(sel)
    shared["sels"] = np.concatenate(sels, axis=1).copy()
    return shared


def _in_maps(inputs):
    x = np.asarray(inputs["x"], np.float32)
    if "shared" not in _cache:
        _cache["shared"] = _shared_maps(inputs)
    shared = _cache["shared"]
    in_maps = []
    for c in range(N_CORES):
        m = dict(shared)
        m["x"] = np.ascontiguousarray(x[c * NS:(c + 1) * NS])
        in_maps.append(m)
    return in_maps


def _make_runner(nc):
    """Cached jitted SPMD executor (run_bass_via_pjrt retraces per call)."""
    import jax
    from jax.experimental.shard_map import shard_map
    from jax.sharding import Mesh, PartitionSpec
    from concourse import bass2jax, mybir as _mb

    bass2jax.install_neuronx_cc_hook()
    partition_name = (nc.partition_id_tensor.name
                      if nc.partition_id_tensor else None)
    in_names, out_names, out_avals, zero_outs = [], [], [], []
    for alloc in nc.m.functions[0].allocations:
        if not isinstance(alloc, _mb.MemoryLocationSet):
            continue
        name = alloc.memorylocations[0].name
        if alloc.kind == "ExternalInput":
            if name != partition_name:
                in_names.append(name)
        elif alloc.kind == "ExternalOutput":
            out_names.append(name)
            shape = tuple(alloc.tensor_shape)
            dtype = _mb.dt.np(alloc.dtype)
            out_avals.append(jax.core.ShapedArray(shape, dtype))
            zero_outs.append(np.zeros((N_CORES * shape[0],) + shape[1:], dtype))
    n_params = len(in_names)
    all_names = in_names + out_names
    if partition_name is not None:
        all_names = all_names + [partition_name]
    donate = tuple(range(n_params, n_params + len(out_names)))

    def _body(*args):
        operands = list(args)
        if partition_name is not None:
            operands.append(bass2jax.partition_id_tensor())
        outs = bass2jax._bass_exec_p.bind(
            *operands, out_avals=tuple(out_avals), in_names=tuple(all_names),
            out_names=tuple(out_names), lowering_input_output_aliases=(),
            sim_require_finite=True, sim_require_nnan=True, nc=nc)
        return tuple(outs)

    devices = jax.devices()[:N_CORES]
    mesh = Mesh(np.asarray(devices), ("core",))
    nin = n_params + len(out_names)
    sharded = jax.jit(
        shard_map(_body, mesh=mesh, in_specs=(PartitionSpec("core"),) * nin,
                  out_specs=(PartitionSpec("core"),) * len(out_names),
                  check_rep=False),
        donate_argnums=donate, keep_unused=True)

    def run(in_maps):
        concat_in = [np.concatenate([np.asarray(in_maps[c][n])
                                     for c in range(N_CORES)], axis=0)
                     for n in in_names]
        out_arrs = sharded(*concat_in, *zero_outs)
        return np.asarray(out_arrs[out_names.index("out")])

    return run


def kernel(**inputs):
    nc = _get_nc()
    if "runner" not in _cache:
        _cache["runner"] = _make_runner(nc)
    return _cache["runner"](_in_maps(inputs))


def profile(**inputs):
    """Run with NTFF tracing; returns (exec_time_ns, results) or None."""
    import types
    try:
        from antenv import axon_hooks
    except ImportError:
        import antenv
        axon_hooks = types.ModuleType("antenv.axon_hooks")
        axon_hooks._hook = None
        def _set(h):
            axon_hooks._hook = h
        def _get():
            return axon_hooks._hook
        axon_hooks.set_axon_ntff_profile_hook = _set
        axon_hooks.get_axon_ntff_profile_hook = _get
        sys.modules["antenv.axon_hooks"] = axon_hooks
        antenv.axon_hooks = axon_hooks
    if axon_hooks.get_axon_ntff_profile_hook() is None:
        sys.path.insert(0, "/root/.axon_site")
        from trn_agent_boot.trn_boot import _ntff_profile_via_ctypes
        hook = _ntff_profile_via_ctypes("/opt/axon/libaxon_pjrt.so")
        if hook is not None:
            axon_hooks.set_axon_ntff_profile_hook(hook)
    import concourse.bass_utils as bu
    bu.upload_artifacts = lambda tmpdir: "(skipped)"
    nc = _get_nc()
    res = run_bass_kernel_spmd(nc, _in_maps(inputs), list(range(N_CORES)),
                               trace=True)
    return res.exec_time_ns


if __name__ == "__main__":
    rng = np.random.default_rng(0)
    fake = {"x": rng.standard_normal((64, 3, 224, 224), dtype=np.float32)}
    for i, (ci, co, kh, kw) in zip(range(1, 7),
                                   [(3, 8, 11, 11), (8, 16, 7, 7), (16, 32, 5, 5),
                                    (32, 32, 5, 5), (32, 64, 5, 5), (64, 64, 3, 3)]):
        fake[f"w{i}"] = rng.standard_normal((co, ci, kh, kw), dtype=np.float32) * 0.1
    for i, c in zip(range(1, 8), [3, 8, 16, 32, 32, 64, 64]):
        fake[f"g{i}"] = np.ones(c, np.float32)
        fake[f"b{i}"] = np.zeros(c, np.float32)
    fake["wfc"] = rng.standard_normal((50, 256), dtype=np.float32) * 0.1
    out = kernel(**fake)
    print("out", out.shape, out[0, :5])
